# revision 37
# baseline (speedup 1.0000x reference)
"""Trainium2 Bass kernel for nn_Attention_32280974197121.

Multi-head attention, N=4096 tokens, E=64 head dim, H=8 heads.
Sharding: one head per NeuronCore (8 cores, no collectives -- the
per-head outputs are combined on the host).

Design (v2) -- dual-engine exp + fp8 DoubleRow attn@v + Wo folded:

  Host packs per head:  wq' = A5*[Wq; bq]  (A5 = 4/ln2, the e5m2
  Schraudolph constant, folded into q so the DVE exp needs no mult),
  wk' = [Wk; bk],  wv' = [Wv @ Wo_h; bv @ Wo_h | e_ones | 0]  (Wo
  folded into v, so attn@v directly accumulates the per-head output
  numerator and the ones column accumulates the softmax denominator).
  All weights and x^T ship as fp16.

  Per core: qT/kT = fp16 projections (PE, fp32 PSUM, stored fp16).
  v blocks -> fp8e4m3, packed per key-chunk PAIR as [128, 2, 66] for
  DoubleRow matmuls.

  Flash loop over 4 query-quarters x 16 key-chunk pairs (256 keys):
    scores  sp = kT_chunk^T @ qT        [128, 1024] PSUM   (PE, fp16)
    exp     alternates per pair between two engines:
      ACT:  et = e4m3( exp(sp/A5 - 3.6) )          (HW exp table)
      DVE:  et = bitcast_e5m2( int8( max(sp,-B5) + B5 ) )
            -- Schraudolph bit-trick exp: sp is A5*score, so
            t = (score-3.6)*A5 + 60 is the e5m2 bit pattern of
            ~exp(score-3.6); max() clamps the (negligible-mass)
            underflow below score ~ -6.8 to +0.0.
      The -3.6 bias keeps e4m3 in range and cancels in softmax.
    attn@v  2 DoubleRow fp8 matmuls per pair (0.5 cyc/col, K=256):
      bacc[66, 1024] += vab[128,2,66]^T (x) et[128,2,512]
      row 64 of bacc = softmax denominator via the ones column.
  Quarter tail: bacc -> SBUF (split ACT/DVE) -> DMA yt (+ rs row).
  Host: out = sum_h yt_h * (SCALE / rs_h) + bo.

  AV matmuls are emitted one pair late so the in-order PE never waits
  on exp; sp PSUM tiles are freed by exp itself (AV reads SBUF et).
  Engine-balance: ~9/16 pairs on ACT, 7/16 on DVE (plus DVE's copy
  background).  PE ~74us, ACT/DVE ~77us theoretical.

Numerics (numpy-sim of this exact scheme): rel err ~7.7e-3 vs the
2e-2 gate (e5m2 Schraudolph ~5.9e-3 alone; e4m3 exact-exp ~4.8e-3).
"""

import numpy as np

N = 4096
E = 64
H = 8
SCALE = 1.0 / E**0.5
NCORES = 8
W = 1024          # n-quarter width
NQ = N // W       # 4 quarters
NS = W // 512     # 512-wide matmul slices per quarter
NJ = N // 128     # 32 key chunks
NP = NJ // 2      # 16 key-chunk pairs (256 keys each)

A5 = 4.0 / np.log(2.0)          # e5m2 Schraudolph scale (folded into wq)
# exp bias: exp(s+EB); cancels in softmax.  Sized so the largest
# per-head score (9.16) stays under IEEE-e4m3's 240 max: e^(9.16-4.1)
# = 158, with ~1.5x margin for fp16 score error.
EB = -4.1
B5 = 60.0 + EB * A5              # e5m2 exponent-bias term (sp domain)

# per-16-pairs exp engine pattern ('A' = ACT exact exp -> e4m3,
# 'D' = DVE Schraudolph -> e5m2); ~9:7 balances ACT vs DVE+copies
PATTERN = ['A', 'D', 'A', 'D', 'A', 'A', 'D', 'A',
           'D', 'A', 'D', 'A', 'A', 'D', 'A', 'D']

_CACHE = {}


def _build_program(reps=1, pattern=None, vab_dt="e4", av_mode="fp8",
                   defer_tail=True, c_r=0.0, exp_split=None):
    """vab_dt: 'e4' (all pairs read e4m3 v), 'e5' (all e5m2), or 'both'
    (A pairs read an e4m3 vab, D pairs an e5m2 vab -- avoids the
    mixed-dtype DoubleRow matmul).  av_mode: 'fp8' (DoubleRow) or
    'bf16' (debug: bf16 et/v, regular matmuls).  exp_split: if set (col
    count), EVERY chunk's exp is split: ACT exact-exps cols [0, split)
    to e5m2 while DVE Schraudolph-exps cols [split, W) -- both engines
    work on every chunk concurrently, the sp tile frees earlier (no PE
    stall on the spool ring), and et is uniformly e5m2."""
    pattern = list(PATTERN if pattern is None else pattern)
    key = ("v2", reps, tuple(pattern), vab_dt, av_mode, defer_tail, c_r,
           exp_split)
    if key in _CACHE:
        return _CACHE[key]

    from contextlib import ExitStack

    import concourse.tile as tile
    from concourse import bacc as bacc_mod, mybir

    f32 = mybir.dt.float32
    f16 = mybir.dt.float16
    f8e4 = mybir.dt.float8e4
    f8e5 = mybir.dt.float8e5
    bf16 = mybir.dt.bfloat16
    i8 = mybir.dt.int8
    i16 = mybir.dt.int16
    Exp = mybir.ActivationFunctionType.Exp
    Max = mybir.AluOpType.max
    Add = mybir.AluOpType.add
    Mult = mybir.AluOpType.mult
    DR = mybir.MatmulPerfMode.DoubleRow

    nc = bacc_mod.Bacc("TRN2", target_bir_lowering=False, debug=False,
                       num_devices=NCORES)

    xt = nc.dram_tensor("xt", [E + 1, N], f16, kind="ExternalInput").ap()
    # packed per-head weights: [wq*A5 | wk | wv_fold + ones col + pad]
    wp = nc.dram_tensor("wp", [E + 1, 3 * E + 2], f16,
                        kind="ExternalInput").ap()
    yt = nc.dram_tensor("yt", [E, N], f32, kind="ExternalOutput").ap()
    rs = nc.dram_tensor("rs", [1, N], f32, kind="ExternalOutput").ap()

    with tile.TileContext(nc) as tc, ExitStack() as ctx:
        rep_loop = (tc.For_i(0, reps, 1) if reps > 1 else None)
        if rep_loop is not None:
            ctx.enter_context(rep_loop)
        const = ctx.enter_context(tc.tile_pool(name="const", bufs=1))
        spool = ctx.enter_context(tc.tile_pool(name="spool", bufs=3,
                                               space="PSUM"))
        bpool = ctx.enter_context(tc.tile_pool(name="bpool", bufs=1,
                                               space="PSUM"))
        epool = ctx.enter_context(tc.tile_pool(name="epool", bufs=4))
        opool = ctx.enter_context(tc.tile_pool(name="opool", bufs=2))

        # warm the ACT exp table before any dependency-carrying work
        scratch = const.tile([1, 1], f32, name="scratch")
        nc.gpsimd.memset(scratch[:], 0.0)
        nc.scalar.activation(scratch[:], scratch[:], Exp)
        # per-partition exp-bias operand for the ACT activations
        ebias = const.tile([128, 1], f32, name="ebias")
        nc.gpsimd.memset(ebias[:], float(EB))

        wp_sb = const.tile([E + 1, 3 * E + 2], f16, name="wp_sb")
        nc.sync.dma_start(wp_sb[:], wp[:])
        wq_sb = wp_sb[:, 0 * E:1 * E]
        wk_sb = wp_sb[:, 1 * E:2 * E]
        wv_sb = wp_sb[:, 2 * E:3 * E + 2]      # (65, 66): ones col + pad
        xt_sb = const.tile([E + 1, N], f16, name="xt_sb")
        # xt chunks on the gpsimd queue so they issue in parallel with
        # the wp DMA on the sync queue
        for c in range(NQ):
            nc.gpsimd.dma_start(xt_sb[:, c * W:(c + 1) * W],
                                xt[:, c * W:(c + 1) * W])

        qt_sb = const.tile([E, N], f16, name="qt_sb")   # A5-scaled q^T
        kt_sb = const.tile([E, N], f16, name="kt_sb")
        # v blocks fp8, pair-major, padded to VBLK=80 bytes per chunk
        # so the DoubleRow ldweights i-stride is 16B-aligned
        # (s3_lw_dual_fp8 ISA restriction); col 64 of each block is the
        # ones column (denominator), cols 66..79 are never read
        VBLK = 80
        vab4 = vab5 = vabb = None
        if av_mode == "bf16":
            vabb = const.tile([128, NP * 2 * VBLK], bf16, name="vabb")
        else:
            if vab_dt in ("e4", "both"):
                vab4 = const.tile([128, NP * 2 * VBLK], f8e4, name="vab4")
            if vab_dt in ("e5", "both"):
                vab5 = const.tile([128, NP * 2 * VBLK], f8e5, name="vab5")

        def vab_for(eng):
            if eng == 'A':
                return vab4 if vab4 is not None else vab5
            return vab5 if vab5 is not None else vab4

        # --- setup helpers (dripped through the first quarters) ---
        def proj_units(c, w_sb, t_sb, nm, use_act_copy=False):
            """3 micro-units: 2 matmuls + 1 PSUM->SBUF fp16 copy."""
            st = {}

            def pp():
                if "pp" not in st:
                    st["pp"] = spool.tile([E, W], f32, tag="s",
                                          name=f"{nm}{c}")
                return st["pp"]

            def mm(s):
                sl = slice(s * 512, (s + 1) * 512)
                xsl = xt_sb[:, c * W + s * 512: c * W + (s + 1) * 512]
                nc.tensor.matmul(pp()[:, sl], w_sb[:], xsl,
                                 start=True, stop=True)

            def cp():
                if use_act_copy:
                    nc.scalar.copy(t_sb[:, c * W:(c + 1) * W], pp()[:])
                else:
                    nc.vector.tensor_copy(t_sb[:, c * W:(c + 1) * W], pp()[:])

            return [lambda: mm(0), lambda: mm(1), cp]

        def v_units(g):
            """2 micro-units covering 4 key-chunks (pairs 2g, 2g+1):
            4 matmuls emitting [v|1] blocks, then 1 copy into vab."""
            st = {}

            def vp():
                if "vp" not in st:
                    st["vp"] = spool.tile([128, 4 * (E + 2)], f32, tag="s",
                                          name=f"vp{g}")
                return st["vp"]

            def mm4():
                for u in range(4):
                    mc = g * 4 + u
                    nc.tensor.matmul(
                        vp()[:, u * (E + 2):(u + 1) * (E + 2)],
                        xt_sb[:, mc * 128:(mc + 1) * 128],
                        wv_sb[:], start=True, stop=True)

            def cp():
                # 4 blocks of 66 strided into the 80-wide padded layout
                src = vp()[:].rearrange("p (b w) -> p b w", w=E + 2)
                for vt in (vab4, vab5, vabb):
                    if vt is None:
                        continue
                    dst = vt[:].rearrange("p (b w) -> p b w", w=VBLK)[
                        :, g * 4:(g + 1) * 4, 0:E + 2]
                    nc.vector.tensor_copy(dst, src)

            return [mm4, cp]

        # chunk 0 of q/k emitted up front at 512 granularity (q copies
        # on ACT, k on DVE), then v groups 0-1 (key chunks 0..7)
        qp0 = spool.tile([E, W], f32, tag="s", name="qp0")
        kp0 = spool.tile([E, W], f32, tag="s", name="kp0")
        for s in range(NS):
            sl = slice(s * 512, (s + 1) * 512)
            xsl = xt_sb[:, s * 512:(s + 1) * 512]
            nc.tensor.matmul(qp0[:, sl], wq_sb[:], xsl, start=True, stop=True)
            nc.tensor.matmul(kp0[:, sl], wk_sb[:], xsl, start=True, stop=True)
            nc.scalar.copy(qt_sb[:, sl], qp0[:, sl])
            nc.vector.tensor_copy(kt_sb[:, sl], kp0[:, sl])
        for u in v_units(0) + v_units(1):
            u()

        # Remaining setup dripped 2 micro-units per pair-slot of quarter
        # 0, with explicit slot alignment so each PSUM staging tile's
        # alloc->copy span stays within the spool ring (<= 2 sp allocs
        # between a pp/vp alloc and its copy).  Deadlines (emission
        # order == Tile dependency order):
        #   kt chunk C needed by scores j=8C, i.e. pair-slot 4C;
        #   v group g (pairs 2g, 2g+1) needed by AV(2g) at slot 2g+1;
        #   qt chunk c needed by quarter c's scores.
        kp1 = proj_units(1, wk_sb, kt_sb, "kp")
        kp2 = proj_units(2, wk_sb, kt_sb, "kp")
        kp3 = proj_units(3, wk_sb, kt_sb, "kp")
        qp1 = proj_units(1, wq_sb, qt_sb, "qp")
        qp2 = proj_units(2, wq_sb, qt_sb, "qp")
        qp3 = proj_units(3, wq_sb, qt_sb, "qp")
        v2, v3, v4, v5 = v_units(2), v_units(3), v_units(4), v_units(5)
        v6, v7 = v_units(6), v_units(7)
        # slot -> units, quarter 0 (slot index = pair t)
        drip0 = {
            1: [kp1[0], kp1[1]],
            2: [kp1[2], v2[0]],
            3: [v2[1], v3[0]],
            4: [v3[1], kp2[0]],
            5: [kp2[1], kp2[2]],
            6: [v4[0], v4[1]],
            7: [v5[0], v5[1]],
            8: [kp3[0], kp3[1]],
            9: [kp3[2], v6[0]],
            10: [v6[1], v7[0]],
            11: [v7[1], qp1[0]],
            12: [qp1[1], qp1[2]],
            13: [qp2[0], qp2[1]],
            14: [qp2[2], qp3[0]],
            15: [qp3[1], qp3[2]],
        }

        # --- main flash-attention loop ---
        hold = {"av": None, "tail": None}
        pair_idx = 0
        for c in range(NQ):
            bst = {}

            def bacc(c=c, bst=bst):
                if "b" not in bst:
                    bst["b"] = bpool.tile([E + 2, W], f32, tag="b",
                                          name=f"b{c}")
                return bst["b"]

            for t in range(NP):
                eng = pattern[pair_idx % len(pattern)]
                pair_idx += 1
                if av_mode == "bf16":
                    et_dt = bf16
                elif exp_split is not None:
                    et_dt = f8e5
                else:
                    et_dt = f8e4
                et = epool.tile([128, 2 * W], et_dt, tag="e",
                                name=f"e{c}_{t}")
                for i in range(2):
                    j = 2 * t + i
                    sp = spool.tile([128, W], f32, tag="s",
                                    name=f"sp{c}_{j}")
                    for s in range(NS):
                        sl = slice(s * 512, (s + 1) * 512)
                        nc.tensor.matmul(
                            sp[:, sl],
                            kt_sb[:, j * 128:(j + 1) * 128],
                            qt_sb[:, c * W + s * 512: c * W + (s + 1) * 512],
                            start=True, stop=True)
                    esl = et[:, i * W:(i + 1) * W]
                    if exp_split is not None and av_mode != "bf16":
                        sa = exp_split
                        nc.scalar.activation(esl[:, 0:sa], sp[:, 0:sa],
                                             Exp, bias=ebias[:],
                                             scale=float(1.0 / A5))
                        nc.vector.tensor_scalar(
                            esl[:, sa:W].bitcast(i8), sp[:, sa:W],
                            float(-B5), float(B5 + c_r), Max, Add)
                    elif eng == 'A':
                        nc.scalar.activation(esl, sp[:], Exp,
                                             bias=ebias[:],
                                             scale=float(1.0 / A5))
                    elif av_mode == "bf16":
                        # bf16 Schraudolph: t = sp*(A7/A5) + B16
                        A7 = 2.0**7 / np.log(2.0)
                        B16 = 16256.0 + EB * A7
                        nc.vector.tensor_scalar(
                            esl.bitcast(i16), sp[:],
                            float(A7 / A5), float(B16), Mult, Add)
                    else:
                        nc.vector.tensor_scalar(
                            esl.bitcast(i8), sp[:],
                            float(-B5), float(B5 + c_r), Max, Add)

                def emit_av(t=t, et=et, eng=eng, bacc=bacc):
                    if av_mode == "bf16":
                        vt_r = vabb[:].rearrange("p (t i m) -> p t i m",
                                                 i=2, m=VBLK)
                        for i in range(2):
                            for h in range(NS):
                                nc.tensor.matmul(
                                    bacc()[:, h * 512:(h + 1) * 512],
                                    vt_r[:, t, i, 0:E + 2],
                                    et[:, i * W + h * 512:
                                       i * W + (h + 1) * 512],
                                    start=(t == 0 and i == 0),
                                    stop=(t == NP - 1 and i == 1))
                        return
                    if exp_split is not None:
                        rhs_t = et[:]          # uniformly e5m2
                    else:
                        rhs_t = et[:] if eng == 'A' else et[:].bitcast(f8e5)
                    rhs_r = rhs_t.rearrange("p (i n) -> p i n", i=2)
                    vt = vab_for(eng)
                    vt_r = vt[:].rearrange("p (t i m) -> p t i m",
                                           i=2, m=VBLK)
                    for h in range(NS):
                        nc.tensor.matmul(
                            bacc()[:, h * 512:(h + 1) * 512],
                            vt_r[:, t, :, 0:E + 2],
                            rhs_r[:, :, h * 512:(h + 1) * 512],
                            start=(t == 0), stop=(t == NP - 1),
                            perf_mode=DR)

                if t == NP - 1:
                    if c < NQ - 1 and not defer_tail:
                        if hold["av"] is not None:
                            hold["av"]()
                            hold["av"] = None
                        emit_av()
                        yo = opool.tile([E + 1, W], f32, tag="y",
                                        name=f"yo{c}")
                        nc.scalar.copy(yo[:, 0:512],
                                       bacc()[0:E + 1, 0:512])
                        nc.vector.tensor_copy(yo[:, 512:1024],
                                              bacc()[0:E + 1, 512:1024])
                        nc.sync.dma_start(yt[:, c * W:(c + 1) * W],
                                          yo[0:E, :])
                        nc.gpsimd.dma_start(rs[0:1, c * W:(c + 1) * W],
                                            yo[E:E + 1, :])
                    elif c < NQ - 1:
                        # flush pair NP-2's deferred AV first
                        if hold["av"] is not None:
                            hold["av"]()
                            hold["av"] = None

                        # defer last AV into the next quarter's pair-0
                        # slot; the bacc->yo copies must be emitted there
                        # too (before pair 1 reallocates the bpool slot),
                        # only the DMAs ride one slot later
                        def make_last(c=c, emit_av=emit_av, bacc=bacc):
                            def last():
                                emit_av()
                                yo = opool.tile([E + 1, W], f32,
                                                tag="y", name=f"yo{c}")
                                nc.scalar.copy(yo[:, 0:512],
                                               bacc()[0:E + 1, 0:512])
                                nc.vector.tensor_copy(
                                    yo[:, 512:1024],
                                    bacc()[0:E + 1, 512:1024])

                                def tail():
                                    nc.sync.dma_start(
                                        yt[:, c * W:(c + 1) * W],
                                        yo[0:E, :])
                                    nc.gpsimd.dma_start(
                                        rs[0:1, c * W:(c + 1) * W],
                                        yo[E:E + 1, :])

                                hold["tail"] = tail
                            return last

                        hold["av"] = make_last()
                    else:
                        # final quarter: emit everything now
                        if hold["av"] is not None:
                            hold["av"]()
                            hold["av"] = None
                        emit_av()
                        yo = opool.tile([E + 1, W], f32, tag="y",
                                        name=f"yo{c}")
                        nc.vector.tensor_copy(yo[:, 0:512],
                                              bacc()[0:E + 1, 0:512])
                        nc.scalar.copy(yo[:, 512:1024],
                                       bacc()[0:E + 1, 512:1024])
                        nc.sync.dma_start(yt[:, c * W:(c + 1) * W],
                                          yo[0:E, :])
                        nc.gpsimd.dma_start(rs[0:1, c * W:(c + 1) * W],
                                            yo[E:E + 1, :])
                else:
                    # AV deferred by one pair so PE never waits on exp
                    if hold["av"] is not None:
                        hold["av"]()
                    hold["av"] = emit_av

                if t == 1 and hold["tail"] is not None:
                    hold["tail"]()
                    hold["tail"] = None
                if c == 0:
                    for u in drip0.get(t, ()):
                        u()

    nc.compile()
    _CACHE[key] = nc
    return nc


def _build_program_v3(reps=1, pattern=None, c_r=0.0):
    """W=512 restructure: 8 query-groups of 512, 1-bank PSUM tiles.

    PSUM: 6 x sp[128,512] (deep score ring, decouples the two exp
    engines) + 2 x bacc[66,512] (double-buffered -- no quarter-boundary
    deferral).  Exp: one instruction per key-chunk per group, engine by
    PATTERN at pair granularity (A-pairs e4m3 exact exp, D-pairs e5m2
    Schraudolph).  attn@v: one DoubleRow matmul per pair per group."""
    pattern = list(PATTERN if pattern is None else pattern)
    key = ("v3", reps, tuple(pattern), c_r)
    if key in _CACHE:
        return _CACHE[key]

    from contextlib import ExitStack

    import concourse.tile as tile
    from concourse import bacc as bacc_mod, mybir

    f32 = mybir.dt.float32
    f16 = mybir.dt.float16
    f8e4 = mybir.dt.float8e4
    f8e5 = mybir.dt.float8e5
    i8 = mybir.dt.int8
    Exp = mybir.ActivationFunctionType.Exp
    Max = mybir.AluOpType.max
    Add = mybir.AluOpType.add
    DR = mybir.MatmulPerfMode.DoubleRow

    G = 512                # group width
    NG = N // G            # 8 groups

    nc = bacc_mod.Bacc("TRN2", target_bir_lowering=False, debug=False,
                       num_devices=NCORES)

    xt = nc.dram_tensor("xt", [E + 1, N], f16, kind="ExternalInput").ap()
    wp = nc.dram_tensor("wp", [E + 1, 3 * E + 2], f16,
                        kind="ExternalInput").ap()
    yt = nc.dram_tensor("yt", [E, N], f32, kind="ExternalOutput").ap()
    rs = nc.dram_tensor("rs", [1, N], f32, kind="ExternalOutput").ap()

    with tile.TileContext(nc) as tc, ExitStack() as ctx:
        rep_loop = (tc.For_i(0, reps, 1) if reps > 1 else None)
        if rep_loop is not None:
            ctx.enter_context(rep_loop)
        const = ctx.enter_context(tc.tile_pool(name="const", bufs=1))
        spool = ctx.enter_context(tc.tile_pool(name="spool", bufs=6,
                                               space="PSUM"))
        bpool = ctx.enter_context(tc.tile_pool(name="bpool", bufs=2,
                                               space="PSUM"))
        epool = ctx.enter_context(tc.tile_pool(name="epool", bufs=6))
        opool = ctx.enter_context(tc.tile_pool(name="opool", bufs=3))

        scratch = const.tile([1, 1], f32, name="scratch")
        nc.gpsimd.memset(scratch[:], 0.0)
        nc.scalar.activation(scratch[:], scratch[:], Exp)
        ebias = const.tile([128, 1], f32, name="ebias")
        nc.gpsimd.memset(ebias[:], float(EB))

        wp_sb = const.tile([E + 1, 3 * E + 2], f16, name="wp_sb")
        nc.sync.dma_start(wp_sb[:], wp[:])
        wq_sb = wp_sb[:, 0 * E:1 * E]
        wk_sb = wp_sb[:, 1 * E:2 * E]
        wv_sb = wp_sb[:, 2 * E:3 * E + 2]
        xt_sb = const.tile([E + 1, N], f16, name="xt_sb")
        for c in range(4):
            nc.gpsimd.dma_start(xt_sb[:, c * 1024:(c + 1) * 1024],
                                xt[:, c * 1024:(c + 1) * 1024])

        qt_sb = const.tile([E, N], f16, name="qt_sb")   # A5-scaled q^T
        kt_sb = const.tile([E, N], f16, name="kt_sb")
        VBLK = 80
        vab = const.tile([128, NP * 2 * VBLK], f8e4, name="vab")
        vab_r = vab[:].rearrange("p (t i m) -> p t i m", i=2, m=VBLK)

        # --- setup helpers: 512-grain units (each mm+cp adjacent) ---
        def proj_units(c, w_sb, t_sb, nm, on_act):
            """4 micro-units: (mm, cp) x 2 halves of a 1024-chunk."""
            units = []
            for hf in range(2):
                st = {}

                def mk(hf=hf, st=st):
                    def mm():
                        st["pp"] = spool.tile([E, G], f32, tag="s",
                                              name=f"{nm}{c}_{hf}")
                        sl = slice(c * 1024 + hf * G, c * 1024 + (hf + 1) * G)
                        nc.tensor.matmul(st["pp"][:], w_sb[:], xt_sb[:, sl],
                                         start=True, stop=True)

                    def cp():
                        sl = slice(c * 1024 + hf * G, c * 1024 + (hf + 1) * G)
                        if on_act:
                            nc.scalar.copy(t_sb[:, sl], st["pp"][:])
                        else:
                            nc.vector.tensor_copy(t_sb[:, sl], st["pp"][:])
                    return [mm, cp]

                units += mk()
            return units

        def v_units(g):
            st = {}

            def mm4():
                st["vp"] = spool.tile([128, 4 * (E + 2)], f32, tag="s",
                                      name=f"vp{g}")
                for u in range(4):
                    mc = g * 4 + u
                    nc.tensor.matmul(
                        st["vp"][:, u * (E + 2):(u + 1) * (E + 2)],
                        xt_sb[:, mc * 128:(mc + 1) * 128],
                        wv_sb[:], start=True, stop=True)

            def cp():
                src = st["vp"][:].rearrange("p (b w) -> p b w", w=E + 2)
                dst = vab[:].rearrange("p (b w) -> p b w", w=VBLK)[
                    :, g * 4:(g + 1) * 4, 0:E + 2]
                nc.vector.tensor_copy(dst, src)

            return [mm4, cp]

        # chunk 0 of q/k up front at 512 grain + v groups 0-1
        for hf in range(2):
            sl = slice(hf * G, (hf + 1) * G)
            qp0 = spool.tile([E, G], f32, tag="s", name=f"qp0_{hf}")
            nc.tensor.matmul(qp0[:], wq_sb[:], xt_sb[:, sl],
                             start=True, stop=True)
            nc.scalar.copy(qt_sb[:, sl], qp0[:])
            kp0 = spool.tile([E, G], f32, tag="s", name=f"kp0_{hf}")
            nc.tensor.matmul(kp0[:], wk_sb[:], xt_sb[:, sl],
                             start=True, stop=True)
            nc.vector.tensor_copy(kt_sb[:, sl], kp0[:])
        for u in v_units(0) + v_units(1):
            u()

        kp1 = proj_units(1, wk_sb, kt_sb, "kp", True)
        kp2 = proj_units(2, wk_sb, kt_sb, "kp", False)
        kp3 = proj_units(3, wk_sb, kt_sb, "kp", True)
        qp1 = proj_units(1, wq_sb, qt_sb, "qp", False)
        qp2 = proj_units(2, wq_sb, qt_sb, "qp", True)
        qp3 = proj_units(3, wq_sb, qt_sb, "qp", False)
        v2, v3, v4, v5 = v_units(2), v_units(3), v_units(4), v_units(5)
        v6, v7 = v_units(6), v_units(7)
        # group-0/1 slot -> units; deadlines: kt chunk C by slot 4C-1,
        # v group g by slot 2g, qt chunk c by group 2c
        drip = {
            (0, 1): kp1[0:2], (0, 2): kp1[2:4],
            (0, 3): v2,
            (0, 4): kp2[0:2], (0, 5): kp2[2:4],
            (0, 6): v3,
            (0, 7): v4,
            (0, 8): kp3[0:2], (0, 9): kp3[2:4],
            (0, 10): v5,
            (0, 11): v6,
            (0, 12): v7,
            (0, 13): qp1[0:2], (0, 14): qp1[2:4],
            (0, 15): qp2[0:2],
            (1, 1): qp2[2:4],
            (1, 3): qp3[0:2],
            (1, 5): qp3[2:4],
        }

        # --- main loop: 8 groups x 16 pairs ---
        hold = {"av": None, "tail": None}
        pair_idx = 0
        for g in range(NG):
            bacc_t = bpool.tile([E + 2, G], f32, tag="b", name=f"b{g}")
            for t in range(NP):
                eng = pattern[pair_idx % len(pattern)]
                pair_idx += 1
                et = epool.tile([128, 2 * G], f8e4, tag="e",
                                name=f"e{g}_{t}")
                for i in range(2):
                    j = 2 * t + i
                    sp = spool.tile([128, G], f32, tag="s",
                                    name=f"sp{g}_{j}")
                    nc.tensor.matmul(
                        sp[:],
                        kt_sb[:, j * 128:(j + 1) * 128],
                        qt_sb[:, g * G:(g + 1) * G],
                        start=True, stop=True)
                    esl = et[:, i * G:(i + 1) * G]
                    if eng == 'A':
                        nc.scalar.activation(esl, sp[:], Exp,
                                             bias=ebias[:],
                                             scale=float(1.0 / A5))
                    else:
                        nc.vector.tensor_scalar(
                            esl.bitcast(i8), sp[:],
                            float(-B5), float(B5 + c_r), Max, Add)

                def emit_av(t=t, et=et, eng=eng, bacc_t=bacc_t):
                    rhs_t = et[:] if eng == 'A' else et[:].bitcast(f8e5)
                    rhs_r = rhs_t.rearrange("p (i n) -> p i n", i=2)
                    nc.tensor.matmul(
                        bacc_t[:], vab_r[:, t, :, 0:E + 2], rhs_r[:],
                        start=(t == 0), stop=(t == NP - 1),
                        perf_mode=DR)

                def make_tail(g=g, bacc_t=bacc_t):
                    def tail():
                        yo = opool.tile([E + 1, G], f32, tag="y",
                                        name=f"yo{g}")
                        if g % 2 == 0:
                            nc.scalar.copy(yo[:], bacc_t[0:E + 1, :])
                        else:
                            nc.vector.tensor_copy(yo[:], bacc_t[0:E + 1, :])
                        nc.sync.dma_start(yt[:, g * G:(g + 1) * G],
                                          yo[0:E, :])
                        nc.gpsimd.dma_start(rs[0:1, g * G:(g + 1) * G],
                                            yo[E:E + 1, :])
                    return tail

                # flush the deferred AV (pair t-1, or the previous
                # group's pair 15 at t==0), then its tail if any; bacc
                # double-buffering lets the new group's AVs overlap
                if hold["av"] is not None:
                    hold["av"]()
                    hold["av"] = None
                if hold["tail"] is not None:
                    hold["tail"]()
                    hold["tail"] = None
                if t < NP - 1:
                    hold["av"] = emit_av
                elif g < NG - 1:
                    hold["av"] = emit_av
                    hold["tail"] = make_tail()
                else:
                    emit_av()
                    make_tail()()

                for u in drip.get((g, t), ()):
                    u()

    nc.compile()
    _CACHE[key] = nc
    return nc


def _run(in_maps, trace=False, trace_cores=None):
    from concourse.bass_utils import run_bass_kernel_spmd

    nc = _build_program()
    return run_bass_kernel_spmd(nc, in_maps, list(range(NCORES)),
                                trace=trace, trace_cores=trace_cores)


def make_in_maps(x, Wq, bq, Wk, bk, Wv, bv, Wo, bo):
    x = np.asarray(x, np.float32)
    Wq, bq = np.asarray(Wq, np.float64), np.asarray(bq, np.float64)
    Wk, bk = np.asarray(Wk, np.float64), np.asarray(bk, np.float64)
    Wv, bv = np.asarray(Wv, np.float64), np.asarray(bv, np.float64)
    Wo = np.asarray(Wo, np.float64)

    xt_aug = np.empty((E + 1, N), np.float16)
    xt_aug[:E] = x.T.astype(np.float16)
    xt_aug[E] = 1.0

    in_maps = []
    for h in range(H):
        Wo_h = Wo[h * E:(h + 1) * E]
        wpack = np.zeros((E + 1, 3 * E + 2), np.float64)
        wpack[:E, 0 * E:1 * E] = Wq[h] * A5
        wpack[E, 0 * E:1 * E] = bq[h] * A5
        wpack[:E, 1 * E:2 * E] = Wk[h]
        wpack[E, 1 * E:2 * E] = bk[h]
        wpack[:E, 2 * E:3 * E] = Wv[h] @ Wo_h
        wpack[E, 2 * E:3 * E] = bv[h] @ Wo_h
        wpack[E, 3 * E] = 1.0            # ones column selector
        in_maps.append({"xt": xt_aug, "wp": wpack.astype(np.float16)})
    return in_maps


def combine_results(results, bo):
    bo = np.asarray(bo, np.float64)
    out = np.zeros((N, E), np.float64)
    for h in range(H):
        yth = results[h]["yt"].astype(np.float64)      # (64, 4096)
        rsh = results[h]["rs"].astype(np.float64)      # (1, 4096)
        out += (yth * (SCALE / rsh)).T
    out += bo
    return out.astype(np.float32)


def kernel(x, Wq, bq, Wk, bk, Wv, bv, Wo, bo):
    in_maps = make_in_maps(x, Wq, bq, Wk, bk, Wv, bv, Wo, bo)
    res = _run(in_maps)
    return combine_results(res.results, bo)


# revision 41
# speedup vs baseline: 1.0172x; 1.0172x over previous
"""Trainium2 Bass kernel for nn_Attention_32280974197121.

Multi-head attention, N=4096 tokens, E=64 head dim, H=8 heads.
Sharding: one head per NeuronCore (8 cores, no collectives -- the
per-head outputs are combined on the host).

Design (v2) -- dual-engine exp + fp8 DoubleRow attn@v + Wo folded:

  Host packs per head:  wq' = A5*[Wq; bq]  (A5 = 4/ln2, the e5m2
  Schraudolph constant, folded into q so the DVE exp needs no mult),
  wk' = [Wk; bk],  wv' = [Wv @ Wo_h; bv @ Wo_h | e_ones | 0]  (Wo
  folded into v, so attn@v directly accumulates the per-head output
  numerator and the ones column accumulates the softmax denominator).
  All weights and x^T ship as fp16.

  Per core: qT/kT = fp16 projections (PE, fp32 PSUM, stored fp16).
  v blocks -> fp8e4m3, packed per key-chunk PAIR as [128, 2, 66] for
  DoubleRow matmuls.

  Flash loop over 4 query-quarters x 16 key-chunk pairs (256 keys):
    scores  sp = kT_chunk^T @ qT        [128, 1024] PSUM   (PE, fp16)
    exp     alternates per pair between two engines:
      ACT:  et = e4m3( exp(sp/A5 - 3.6) )          (HW exp table)
      DVE:  et = bitcast_e5m2( int8( max(sp,-B5) + B5 ) )
            -- Schraudolph bit-trick exp: sp is A5*score, so
            t = (score-3.6)*A5 + 60 is the e5m2 bit pattern of
            ~exp(score-3.6); max() clamps the (negligible-mass)
            underflow below score ~ -6.8 to +0.0.
      The -3.6 bias keeps e4m3 in range and cancels in softmax.
    attn@v  2 DoubleRow fp8 matmuls per pair (0.5 cyc/col, K=256):
      bacc[66, 1024] += vab[128,2,66]^T (x) et[128,2,512]
      row 64 of bacc = softmax denominator via the ones column.
  Quarter tail: bacc -> SBUF (split ACT/DVE) -> DMA yt (+ rs row).
  Host: out = sum_h yt_h * (SCALE / rs_h) + bo.

  AV matmuls are emitted one pair late so the in-order PE never waits
  on exp; sp PSUM tiles are freed by exp itself (AV reads SBUF et).
  Engine-balance: ~9/16 pairs on ACT, 7/16 on DVE (plus DVE's copy
  background).  PE ~74us, ACT/DVE ~77us theoretical.

Numerics (numpy-sim of this exact scheme): rel err ~7.7e-3 vs the
2e-2 gate (e5m2 Schraudolph ~5.9e-3 alone; e4m3 exact-exp ~4.8e-3).
"""

import numpy as np

N = 4096
E = 64
H = 8
SCALE = 1.0 / E**0.5
NCORES = 8
W = 1024          # n-quarter width
NQ = N // W       # 4 quarters
NS = W // 512     # 512-wide matmul slices per quarter
NJ = N // 128     # 32 key chunks
NP = NJ // 2      # 16 key-chunk pairs (256 keys each)

A5 = 4.0 / np.log(2.0)          # e5m2 Schraudolph scale (folded into wq)
# exp bias: exp(s+EB); cancels in softmax.  Sized so the largest
# per-head score (9.16) stays under IEEE-e4m3's 240 max: e^(9.16-4.1)
# = 158, with ~1.5x margin for fp16 score error.
EB = -4.1
B5 = 60.0 + EB * A5              # e5m2 exponent-bias term (sp domain)

# per-pair exp engine pattern ('A' = ACT exact exp -> e4m3, 'D' = DVE
# Schraudolph -> e5m2); 17:15 over 32 pairs balances ACT vs DVE+copies
# (timeline-sim sweep: 17:15 beats 16:16 and 18:14)
def _mk_pattern(nA, nD):
    pat, a, d = [], 0, 0
    for _ in range(nA + nD):
        if a * nD <= d * nA and a < nA:
            pat.append('A')
            a += 1
        else:
            pat.append('D')
            d += 1
    return pat


PATTERN = _mk_pattern(17, 15)

_CACHE = {}


def _build_program_v2(reps=1, pattern=None, vab_dt="e4", av_mode="fp8",
                      defer_tail=True, c_r=0.0, exp_split=None):
    """vab_dt: 'e4' (all pairs read e4m3 v), 'e5' (all e5m2), or 'both'
    (A pairs read an e4m3 vab, D pairs an e5m2 vab -- avoids the
    mixed-dtype DoubleRow matmul).  av_mode: 'fp8' (DoubleRow) or
    'bf16' (debug: bf16 et/v, regular matmuls).  exp_split: if set (col
    count), EVERY chunk's exp is split: ACT exact-exps cols [0, split)
    to e5m2 while DVE Schraudolph-exps cols [split, W) -- both engines
    work on every chunk concurrently, the sp tile frees earlier (no PE
    stall on the spool ring), and et is uniformly e5m2."""
    pattern = list(PATTERN if pattern is None else pattern)
    key = ("v2", reps, tuple(pattern), vab_dt, av_mode, defer_tail, c_r,
           exp_split)
    if key in _CACHE:
        return _CACHE[key]

    from contextlib import ExitStack

    import concourse.tile as tile
    from concourse import bacc as bacc_mod, mybir

    f32 = mybir.dt.float32
    f16 = mybir.dt.float16
    f8e4 = mybir.dt.float8e4
    f8e5 = mybir.dt.float8e5
    bf16 = mybir.dt.bfloat16
    i8 = mybir.dt.int8
    i16 = mybir.dt.int16
    Exp = mybir.ActivationFunctionType.Exp
    Max = mybir.AluOpType.max
    Add = mybir.AluOpType.add
    Mult = mybir.AluOpType.mult
    DR = mybir.MatmulPerfMode.DoubleRow

    nc = bacc_mod.Bacc("TRN2", target_bir_lowering=False, debug=False,
                       num_devices=NCORES)

    xt = nc.dram_tensor("xt", [E + 1, N], f16, kind="ExternalInput").ap()
    # packed per-head weights: [wq*A5 | wk | wv_fold + ones col + pad]
    wp = nc.dram_tensor("wp", [E + 1, 3 * E + 2], f16,
                        kind="ExternalInput").ap()
    yt = nc.dram_tensor("yt", [E, N], f32, kind="ExternalOutput").ap()
    rs = nc.dram_tensor("rs", [1, N], f32, kind="ExternalOutput").ap()

    with tile.TileContext(nc) as tc, ExitStack() as ctx:
        rep_loop = (tc.For_i(0, reps, 1) if reps > 1 else None)
        if rep_loop is not None:
            ctx.enter_context(rep_loop)
        const = ctx.enter_context(tc.tile_pool(name="const", bufs=1))
        spool = ctx.enter_context(tc.tile_pool(name="spool", bufs=3,
                                               space="PSUM"))
        bpool = ctx.enter_context(tc.tile_pool(name="bpool", bufs=1,
                                               space="PSUM"))
        epool = ctx.enter_context(tc.tile_pool(name="epool", bufs=4))
        opool = ctx.enter_context(tc.tile_pool(name="opool", bufs=2))

        # warm the ACT exp table before any dependency-carrying work
        scratch = const.tile([1, 1], f32, name="scratch")
        nc.gpsimd.memset(scratch[:], 0.0)
        nc.scalar.activation(scratch[:], scratch[:], Exp)
        # per-partition exp-bias operand for the ACT activations
        ebias = const.tile([128, 1], f32, name="ebias")
        nc.gpsimd.memset(ebias[:], float(EB))

        wp_sb = const.tile([E + 1, 3 * E + 2], f16, name="wp_sb")
        nc.sync.dma_start(wp_sb[:], wp[:])
        wq_sb = wp_sb[:, 0 * E:1 * E]
        wk_sb = wp_sb[:, 1 * E:2 * E]
        wv_sb = wp_sb[:, 2 * E:3 * E + 2]      # (65, 66): ones col + pad
        xt_sb = const.tile([E + 1, N], f16, name="xt_sb")
        # xt chunks on the gpsimd queue so they issue in parallel with
        # the wp DMA on the sync queue
        for c in range(NQ):
            nc.gpsimd.dma_start(xt_sb[:, c * W:(c + 1) * W],
                                xt[:, c * W:(c + 1) * W])

        qt_sb = const.tile([E, N], f16, name="qt_sb")   # A5-scaled q^T
        kt_sb = const.tile([E, N], f16, name="kt_sb")
        # v blocks fp8, pair-major, padded to VBLK=80 bytes per chunk
        # so the DoubleRow ldweights i-stride is 16B-aligned
        # (s3_lw_dual_fp8 ISA restriction); col 64 of each block is the
        # ones column (denominator), cols 66..79 are never read
        VBLK = 80
        vab4 = vab5 = vabb = None
        if av_mode == "bf16":
            vabb = const.tile([128, NP * 2 * VBLK], bf16, name="vabb")
        else:
            if vab_dt in ("e4", "both"):
                vab4 = const.tile([128, NP * 2 * VBLK], f8e4, name="vab4")
            if vab_dt in ("e5", "both"):
                vab5 = const.tile([128, NP * 2 * VBLK], f8e5, name="vab5")

        def vab_for(eng):
            if eng == 'A':
                return vab4 if vab4 is not None else vab5
            return vab5 if vab5 is not None else vab4

        # --- setup helpers (dripped through the first quarters) ---
        def proj_units(c, w_sb, t_sb, nm, use_act_copy=False):
            """3 micro-units: 2 matmuls + 1 PSUM->SBUF fp16 copy."""
            st = {}

            def pp():
                if "pp" not in st:
                    st["pp"] = spool.tile([E, W], f32, tag="s",
                                          name=f"{nm}{c}")
                return st["pp"]

            def mm(s):
                sl = slice(s * 512, (s + 1) * 512)
                xsl = xt_sb[:, c * W + s * 512: c * W + (s + 1) * 512]
                nc.tensor.matmul(pp()[:, sl], w_sb[:], xsl,
                                 start=True, stop=True)

            def cp():
                if use_act_copy:
                    nc.scalar.copy(t_sb[:, c * W:(c + 1) * W], pp()[:])
                else:
                    nc.vector.tensor_copy(t_sb[:, c * W:(c + 1) * W], pp()[:])

            return [lambda: mm(0), lambda: mm(1), cp]

        def v_units(g):
            """2 micro-units covering 4 key-chunks (pairs 2g, 2g+1):
            4 matmuls emitting [v|1] blocks, then 1 copy into vab."""
            st = {}

            def vp():
                if "vp" not in st:
                    st["vp"] = spool.tile([128, 4 * (E + 2)], f32, tag="s",
                                          name=f"vp{g}")
                return st["vp"]

            def mm4():
                for u in range(4):
                    mc = g * 4 + u
                    nc.tensor.matmul(
                        vp()[:, u * (E + 2):(u + 1) * (E + 2)],
                        xt_sb[:, mc * 128:(mc + 1) * 128],
                        wv_sb[:], start=True, stop=True)

            def cp():
                # 4 blocks of 66 strided into the 80-wide padded layout
                src = vp()[:].rearrange("p (b w) -> p b w", w=E + 2)
                for vt in (vab4, vab5, vabb):
                    if vt is None:
                        continue
                    dst = vt[:].rearrange("p (b w) -> p b w", w=VBLK)[
                        :, g * 4:(g + 1) * 4, 0:E + 2]
                    nc.vector.tensor_copy(dst, src)

            return [mm4, cp]

        # chunk 0 of q/k emitted up front at 512 granularity (q copies
        # on ACT, k on DVE), then v groups 0-1 (key chunks 0..7)
        qp0 = spool.tile([E, W], f32, tag="s", name="qp0")
        kp0 = spool.tile([E, W], f32, tag="s", name="kp0")
        for s in range(NS):
            sl = slice(s * 512, (s + 1) * 512)
            xsl = xt_sb[:, s * 512:(s + 1) * 512]
            nc.tensor.matmul(qp0[:, sl], wq_sb[:], xsl, start=True, stop=True)
            nc.tensor.matmul(kp0[:, sl], wk_sb[:], xsl, start=True, stop=True)
            nc.scalar.copy(qt_sb[:, sl], qp0[:, sl])
            nc.vector.tensor_copy(kt_sb[:, sl], kp0[:, sl])
        for u in v_units(0) + v_units(1):
            u()

        # Remaining setup dripped 2 micro-units per pair-slot of quarter
        # 0, with explicit slot alignment so each PSUM staging tile's
        # alloc->copy span stays within the spool ring (<= 2 sp allocs
        # between a pp/vp alloc and its copy).  Deadlines (emission
        # order == Tile dependency order):
        #   kt chunk C needed by scores j=8C, i.e. pair-slot 4C;
        #   v group g (pairs 2g, 2g+1) needed by AV(2g) at slot 2g+1;
        #   qt chunk c needed by quarter c's scores.
        kp1 = proj_units(1, wk_sb, kt_sb, "kp")
        kp2 = proj_units(2, wk_sb, kt_sb, "kp")
        kp3 = proj_units(3, wk_sb, kt_sb, "kp")
        qp1 = proj_units(1, wq_sb, qt_sb, "qp")
        qp2 = proj_units(2, wq_sb, qt_sb, "qp")
        qp3 = proj_units(3, wq_sb, qt_sb, "qp")
        v2, v3, v4, v5 = v_units(2), v_units(3), v_units(4), v_units(5)
        v6, v7 = v_units(6), v_units(7)
        # slot -> units, quarter 0 (slot index = pair t)
        drip0 = {
            1: [kp1[0], kp1[1]],
            2: [kp1[2], v2[0]],
            3: [v2[1], v3[0]],
            4: [v3[1], kp2[0]],
            5: [kp2[1], kp2[2]],
            6: [v4[0], v4[1]],
            7: [v5[0], v5[1]],
            8: [kp3[0], kp3[1]],
            9: [kp3[2], v6[0]],
            10: [v6[1], v7[0]],
            11: [v7[1], qp1[0]],
            12: [qp1[1], qp1[2]],
            13: [qp2[0], qp2[1]],
            14: [qp2[2], qp3[0]],
            15: [qp3[1], qp3[2]],
        }

        # --- main flash-attention loop ---
        hold = {"av": None, "tail": None}
        pair_idx = 0
        for c in range(NQ):
            bst = {}

            def bacc(c=c, bst=bst):
                if "b" not in bst:
                    bst["b"] = bpool.tile([E + 2, W], f32, tag="b",
                                          name=f"b{c}")
                return bst["b"]

            for t in range(NP):
                eng = pattern[pair_idx % len(pattern)]
                pair_idx += 1
                if av_mode == "bf16":
                    et_dt = bf16
                elif exp_split is not None:
                    et_dt = f8e5
                else:
                    et_dt = f8e4
                et = epool.tile([128, 2 * W], et_dt, tag="e",
                                name=f"e{c}_{t}")
                for i in range(2):
                    j = 2 * t + i
                    sp = spool.tile([128, W], f32, tag="s",
                                    name=f"sp{c}_{j}")
                    for s in range(NS):
                        sl = slice(s * 512, (s + 1) * 512)
                        nc.tensor.matmul(
                            sp[:, sl],
                            kt_sb[:, j * 128:(j + 1) * 128],
                            qt_sb[:, c * W + s * 512: c * W + (s + 1) * 512],
                            start=True, stop=True)
                    esl = et[:, i * W:(i + 1) * W]
                    if exp_split is not None and av_mode != "bf16":
                        sa = exp_split
                        nc.scalar.activation(esl[:, 0:sa], sp[:, 0:sa],
                                             Exp, bias=ebias[:],
                                             scale=float(1.0 / A5))
                        nc.vector.tensor_scalar(
                            esl[:, sa:W].bitcast(i8), sp[:, sa:W],
                            float(-B5), float(B5 + c_r), Max, Add)
                    elif eng == 'A':
                        nc.scalar.activation(esl, sp[:], Exp,
                                             bias=ebias[:],
                                             scale=float(1.0 / A5))
                    elif av_mode == "bf16":
                        # bf16 Schraudolph: t = sp*(A7/A5) + B16
                        A7 = 2.0**7 / np.log(2.0)
                        B16 = 16256.0 + EB * A7
                        nc.vector.tensor_scalar(
                            esl.bitcast(i16), sp[:],
                            float(A7 / A5), float(B16), Mult, Add)
                    else:
                        nc.vector.tensor_scalar(
                            esl.bitcast(i8), sp[:],
                            float(-B5), float(B5 + c_r), Max, Add)

                def emit_av(t=t, et=et, eng=eng, bacc=bacc):
                    if av_mode == "bf16":
                        vt_r = vabb[:].rearrange("p (t i m) -> p t i m",
                                                 i=2, m=VBLK)
                        for i in range(2):
                            for h in range(NS):
                                nc.tensor.matmul(
                                    bacc()[:, h * 512:(h + 1) * 512],
                                    vt_r[:, t, i, 0:E + 2],
                                    et[:, i * W + h * 512:
                                       i * W + (h + 1) * 512],
                                    start=(t == 0 and i == 0),
                                    stop=(t == NP - 1 and i == 1))
                        return
                    if exp_split is not None:
                        rhs_t = et[:]          # uniformly e5m2
                    else:
                        rhs_t = et[:] if eng == 'A' else et[:].bitcast(f8e5)
                    rhs_r = rhs_t.rearrange("p (i n) -> p i n", i=2)
                    vt = vab_for(eng)
                    vt_r = vt[:].rearrange("p (t i m) -> p t i m",
                                           i=2, m=VBLK)
                    for h in range(NS):
                        nc.tensor.matmul(
                            bacc()[:, h * 512:(h + 1) * 512],
                            vt_r[:, t, :, 0:E + 2],
                            rhs_r[:, :, h * 512:(h + 1) * 512],
                            start=(t == 0), stop=(t == NP - 1),
                            perf_mode=DR)

                if t == NP - 1:
                    if c < NQ - 1 and not defer_tail:
                        if hold["av"] is not None:
                            hold["av"]()
                            hold["av"] = None
                        emit_av()
                        yo = opool.tile([E + 1, W], f32, tag="y",
                                        name=f"yo{c}")
                        nc.scalar.copy(yo[:, 0:512],
                                       bacc()[0:E + 1, 0:512])
                        nc.vector.tensor_copy(yo[:, 512:1024],
                                              bacc()[0:E + 1, 512:1024])
                        nc.sync.dma_start(yt[:, c * W:(c + 1) * W],
                                          yo[0:E, :])
                        nc.gpsimd.dma_start(rs[0:1, c * W:(c + 1) * W],
                                            yo[E:E + 1, :])
                    elif c < NQ - 1:
                        # flush pair NP-2's deferred AV first
                        if hold["av"] is not None:
                            hold["av"]()
                            hold["av"] = None

                        # defer last AV into the next quarter's pair-0
                        # slot; the bacc->yo copies must be emitted there
                        # too (before pair 1 reallocates the bpool slot),
                        # only the DMAs ride one slot later
                        def make_last(c=c, emit_av=emit_av, bacc=bacc):
                            def last():
                                emit_av()
                                yo = opool.tile([E + 1, W], f32,
                                                tag="y", name=f"yo{c}")
                                nc.scalar.copy(yo[:, 0:512],
                                               bacc()[0:E + 1, 0:512])
                                nc.vector.tensor_copy(
                                    yo[:, 512:1024],
                                    bacc()[0:E + 1, 512:1024])

                                def tail():
                                    nc.sync.dma_start(
                                        yt[:, c * W:(c + 1) * W],
                                        yo[0:E, :])
                                    nc.gpsimd.dma_start(
                                        rs[0:1, c * W:(c + 1) * W],
                                        yo[E:E + 1, :])

                                hold["tail"] = tail
                            return last

                        hold["av"] = make_last()
                    else:
                        # final quarter: emit everything now
                        if hold["av"] is not None:
                            hold["av"]()
                            hold["av"] = None
                        emit_av()
                        yo = opool.tile([E + 1, W], f32, tag="y",
                                        name=f"yo{c}")
                        nc.vector.tensor_copy(yo[:, 0:512],
                                              bacc()[0:E + 1, 0:512])
                        nc.scalar.copy(yo[:, 512:1024],
                                       bacc()[0:E + 1, 512:1024])
                        nc.sync.dma_start(yt[:, c * W:(c + 1) * W],
                                          yo[0:E, :])
                        nc.gpsimd.dma_start(rs[0:1, c * W:(c + 1) * W],
                                            yo[E:E + 1, :])
                else:
                    # AV deferred by one pair so PE never waits on exp
                    if hold["av"] is not None:
                        hold["av"]()
                    hold["av"] = emit_av

                if t == 1 and hold["tail"] is not None:
                    hold["tail"]()
                    hold["tail"] = None
                if c == 0:
                    for u in drip0.get(t, ()):
                        u()

    nc.compile()
    _CACHE[key] = nc
    return nc


def _build_program_v3(reps=1, pattern=None, c_r=0.0):
    """W=512 restructure: 8 query-groups of 512, 1-bank PSUM tiles.

    PSUM: 6 x sp[128,512] (deep score ring, decouples the two exp
    engines) + 2 x bacc[66,512] (double-buffered -- no quarter-boundary
    deferral).  Exp: one instruction per key-chunk per group, engine by
    PATTERN at pair granularity (A-pairs e4m3 exact exp, D-pairs e5m2
    Schraudolph).  attn@v: one DoubleRow matmul per pair per group."""
    pattern = list(PATTERN if pattern is None else pattern)
    key = ("v3", reps, tuple(pattern), c_r)
    if key in _CACHE:
        return _CACHE[key]

    from contextlib import ExitStack

    import concourse.tile as tile
    from concourse import bacc as bacc_mod, mybir

    f32 = mybir.dt.float32
    f16 = mybir.dt.float16
    f8e4 = mybir.dt.float8e4
    f8e5 = mybir.dt.float8e5
    i8 = mybir.dt.int8
    Exp = mybir.ActivationFunctionType.Exp
    Max = mybir.AluOpType.max
    Add = mybir.AluOpType.add
    DR = mybir.MatmulPerfMode.DoubleRow

    G = 512                # group width
    NG = N // G            # 8 groups

    nc = bacc_mod.Bacc("TRN2", target_bir_lowering=False, debug=False,
                       num_devices=NCORES)

    xt = nc.dram_tensor("xt", [E + 1, N], f16, kind="ExternalInput").ap()
    wp = nc.dram_tensor("wp", [E + 1, 3 * E + 2], f16,
                        kind="ExternalInput").ap()
    yt = nc.dram_tensor("yt", [E, N], f32, kind="ExternalOutput").ap()
    rs = nc.dram_tensor("rs", [1, N], f32, kind="ExternalOutput").ap()

    with tile.TileContext(nc) as tc, ExitStack() as ctx:
        rep_loop = (tc.For_i(0, reps, 1) if reps > 1 else None)
        if rep_loop is not None:
            ctx.enter_context(rep_loop)
        const = ctx.enter_context(tc.tile_pool(name="const", bufs=1))
        spool = ctx.enter_context(tc.tile_pool(name="spool", bufs=6,
                                               space="PSUM"))
        bpool = ctx.enter_context(tc.tile_pool(name="bpool", bufs=2,
                                               space="PSUM"))
        epool = ctx.enter_context(tc.tile_pool(name="epool", bufs=6))
        opool = ctx.enter_context(tc.tile_pool(name="opool", bufs=3))

        scratch = const.tile([1, 1], f32, name="scratch")
        nc.gpsimd.memset(scratch[:], 0.0)
        nc.scalar.activation(scratch[:], scratch[:], Exp)
        ebias = const.tile([128, 1], f32, name="ebias")
        nc.gpsimd.memset(ebias[:], float(EB))

        wp_sb = const.tile([E + 1, 3 * E + 2], f16, name="wp_sb")
        nc.sync.dma_start(wp_sb[:], wp[:])
        wq_sb = wp_sb[:, 0 * E:1 * E]
        wk_sb = wp_sb[:, 1 * E:2 * E]
        wv_sb = wp_sb[:, 2 * E:3 * E + 2]
        xt_sb = const.tile([E + 1, N], f16, name="xt_sb")
        for c in range(4):
            nc.gpsimd.dma_start(xt_sb[:, c * 1024:(c + 1) * 1024],
                                xt[:, c * 1024:(c + 1) * 1024])

        qt_sb = const.tile([E, N], f16, name="qt_sb")   # A5-scaled q^T
        kt_sb = const.tile([E, N], f16, name="kt_sb")
        VBLK = 80
        vab = const.tile([128, NP * 2 * VBLK], f8e4, name="vab")
        vab_r = vab[:].rearrange("p (t i m) -> p t i m", i=2, m=VBLK)

        # --- setup helpers: 512-grain units (each mm+cp adjacent) ---
        def proj_units(c, w_sb, t_sb, nm, on_act):
            """4 micro-units: (mm, cp) x 2 halves of a 1024-chunk."""
            units = []
            for hf in range(2):
                st = {}

                def mk(hf=hf, st=st):
                    def mm():
                        st["pp"] = spool.tile([E, G], f32, tag="s",
                                              name=f"{nm}{c}_{hf}")
                        sl = slice(c * 1024 + hf * G, c * 1024 + (hf + 1) * G)
                        nc.tensor.matmul(st["pp"][:], w_sb[:], xt_sb[:, sl],
                                         start=True, stop=True)

                    def cp():
                        sl = slice(c * 1024 + hf * G, c * 1024 + (hf + 1) * G)
                        if on_act:
                            nc.scalar.copy(t_sb[:, sl], st["pp"][:])
                        else:
                            nc.vector.tensor_copy(t_sb[:, sl], st["pp"][:])
                    return [mm, cp]

                units += mk()
            return units

        def v_units(g):
            st = {}

            def mm4():
                st["vp"] = spool.tile([128, 4 * (E + 2)], f32, tag="s",
                                      name=f"vp{g}")
                for u in range(4):
                    mc = g * 4 + u
                    nc.tensor.matmul(
                        st["vp"][:, u * (E + 2):(u + 1) * (E + 2)],
                        xt_sb[:, mc * 128:(mc + 1) * 128],
                        wv_sb[:], start=True, stop=True)

            def cp():
                src = st["vp"][:].rearrange("p (b w) -> p b w", w=E + 2)
                dst = vab[:].rearrange("p (b w) -> p b w", w=VBLK)[
                    :, g * 4:(g + 1) * 4, 0:E + 2]
                nc.vector.tensor_copy(dst, src)

            return [mm4, cp]

        # chunk 0 of q/k up front at 512 grain + v groups 0-1
        for hf in range(2):
            sl = slice(hf * G, (hf + 1) * G)
            qp0 = spool.tile([E, G], f32, tag="s", name=f"qp0_{hf}")
            nc.tensor.matmul(qp0[:], wq_sb[:], xt_sb[:, sl],
                             start=True, stop=True)
            nc.scalar.copy(qt_sb[:, sl], qp0[:])
            kp0 = spool.tile([E, G], f32, tag="s", name=f"kp0_{hf}")
            nc.tensor.matmul(kp0[:], wk_sb[:], xt_sb[:, sl],
                             start=True, stop=True)
            nc.vector.tensor_copy(kt_sb[:, sl], kp0[:])
        for u in v_units(0) + v_units(1):
            u()

        kp1 = proj_units(1, wk_sb, kt_sb, "kp", True)
        kp2 = proj_units(2, wk_sb, kt_sb, "kp", False)
        kp3 = proj_units(3, wk_sb, kt_sb, "kp", True)
        qp1 = proj_units(1, wq_sb, qt_sb, "qp", False)
        qp2 = proj_units(2, wq_sb, qt_sb, "qp", True)
        qp3 = proj_units(3, wq_sb, qt_sb, "qp", False)
        v2, v3, v4, v5 = v_units(2), v_units(3), v_units(4), v_units(5)
        v6, v7 = v_units(6), v_units(7)
        # group-0/1 slot -> units; deadlines: kt chunk C by slot 4C-1,
        # v group g by slot 2g, qt chunk c by group 2c
        drip = {
            (0, 1): kp1[0:2], (0, 2): kp1[2:4],
            (0, 3): v2,
            (0, 4): kp2[0:2], (0, 5): kp2[2:4],
            (0, 6): v3,
            (0, 7): v4,
            (0, 8): kp3[0:2], (0, 9): kp3[2:4],
            (0, 10): v5,
            (0, 11): v6,
            (0, 12): v7,
            (0, 13): qp1[0:2], (0, 14): qp1[2:4],
            (0, 15): qp2[0:2],
            (1, 1): qp2[2:4],
            (1, 3): qp3[0:2],
            (1, 5): qp3[2:4],
        }

        # --- main loop: 8 groups x 16 pairs ---
        hold = {"av": None, "tail": None}
        pair_idx = 0
        for g in range(NG):
            bacc_t = bpool.tile([E + 2, G], f32, tag="b", name=f"b{g}")
            for t in range(NP):
                eng = pattern[pair_idx % len(pattern)]
                pair_idx += 1
                et = epool.tile([128, 2 * G], f8e4, tag="e",
                                name=f"e{g}_{t}")
                for i in range(2):
                    j = 2 * t + i
                    sp = spool.tile([128, G], f32, tag="s",
                                    name=f"sp{g}_{j}")
                    nc.tensor.matmul(
                        sp[:],
                        kt_sb[:, j * 128:(j + 1) * 128],
                        qt_sb[:, g * G:(g + 1) * G],
                        start=True, stop=True)
                    esl = et[:, i * G:(i + 1) * G]
                    if eng == 'A':
                        nc.scalar.activation(esl, sp[:], Exp,
                                             bias=ebias[:],
                                             scale=float(1.0 / A5))
                    else:
                        nc.vector.tensor_scalar(
                            esl.bitcast(i8), sp[:],
                            float(-B5), float(B5 + c_r), Max, Add)

                def emit_av(t=t, et=et, eng=eng, bacc_t=bacc_t):
                    rhs_t = et[:] if eng == 'A' else et[:].bitcast(f8e5)
                    rhs_r = rhs_t.rearrange("p (i n) -> p i n", i=2)
                    nc.tensor.matmul(
                        bacc_t[:], vab_r[:, t, :, 0:E + 2], rhs_r[:],
                        start=(t == 0), stop=(t == NP - 1),
                        perf_mode=DR)

                def make_tail(g=g, bacc_t=bacc_t):
                    def tail():
                        yo = opool.tile([E + 1, G], f32, tag="y",
                                        name=f"yo{g}")
                        if g % 2 == 0:
                            nc.scalar.copy(yo[:], bacc_t[0:E + 1, :])
                        else:
                            nc.vector.tensor_copy(yo[:], bacc_t[0:E + 1, :])
                        nc.sync.dma_start(yt[:, g * G:(g + 1) * G],
                                          yo[0:E, :])
                        nc.gpsimd.dma_start(rs[0:1, g * G:(g + 1) * G],
                                            yo[E:E + 1, :])
                    return tail

                # flush the deferred AV (pair t-1, or the previous
                # group's pair 15 at t==0), then its tail if any; bacc
                # double-buffering lets the new group's AVs overlap
                if hold["av"] is not None:
                    hold["av"]()
                    hold["av"] = None
                if hold["tail"] is not None:
                    hold["tail"]()
                    hold["tail"] = None
                if t < NP - 1:
                    hold["av"] = emit_av
                elif g < NG - 1:
                    hold["av"] = emit_av
                    hold["tail"] = make_tail()
                else:
                    emit_av()
                    make_tail()()

                for u in drip.get((g, t), ()):
                    u()

    nc.compile()
    _CACHE[key] = nc
    return nc


def _build_program(reps=1, **kw):
    return _build_program_v3(reps=reps, **kw)


def _run(in_maps, trace=False, trace_cores=None):
    from concourse.bass_utils import run_bass_kernel_spmd

    nc = _build_program()
    return run_bass_kernel_spmd(nc, in_maps, list(range(NCORES)),
                                trace=trace, trace_cores=trace_cores)


def make_in_maps(x, Wq, bq, Wk, bk, Wv, bv, Wo, bo):
    x = np.asarray(x, np.float32)
    Wq, bq = np.asarray(Wq, np.float64), np.asarray(bq, np.float64)
    Wk, bk = np.asarray(Wk, np.float64), np.asarray(bk, np.float64)
    Wv, bv = np.asarray(Wv, np.float64), np.asarray(bv, np.float64)
    Wo = np.asarray(Wo, np.float64)

    xt_aug = np.empty((E + 1, N), np.float16)
    xt_aug[:E] = x.T.astype(np.float16)
    xt_aug[E] = 1.0

    in_maps = []
    for h in range(H):
        Wo_h = Wo[h * E:(h + 1) * E]
        wpack = np.zeros((E + 1, 3 * E + 2), np.float64)
        wpack[:E, 0 * E:1 * E] = Wq[h] * A5
        wpack[E, 0 * E:1 * E] = bq[h] * A5
        wpack[:E, 1 * E:2 * E] = Wk[h]
        wpack[E, 1 * E:2 * E] = bk[h]
        wpack[:E, 2 * E:3 * E] = Wv[h] @ Wo_h
        wpack[E, 2 * E:3 * E] = bv[h] @ Wo_h
        wpack[E, 3 * E] = 1.0            # ones column selector
        in_maps.append({"xt": xt_aug, "wp": wpack.astype(np.float16)})
    return in_maps


def combine_results(results, bo):
    bo = np.asarray(bo, np.float64)
    out = np.zeros((N, E), np.float64)
    for h in range(H):
        yth = results[h]["yt"].astype(np.float64)      # (64, 4096)
        rsh = results[h]["rs"].astype(np.float64)      # (1, 4096)
        out += (yth * (SCALE / rsh)).T
    out += bo
    return out.astype(np.float32)


def kernel(x, Wq, bq, Wk, bk, Wv, bv, Wo, bo):
    in_maps = make_in_maps(x, Wq, bq, Wk, bk, Wv, bv, Wo, bo)
    res = _run(in_maps)
    return combine_results(res.results, bo)


# revision 54
# speedup vs baseline: 1.4191x; 1.3951x over previous
"""Trainium2 Bass kernel for nn_Attention_32280974197121.

Multi-head attention, N=4096 tokens, E=64 head dim, H=8 heads.
Sharding: one head per NeuronCore (8 cores, no collectives -- the
per-head outputs are combined on the host).

Design (v2) -- dual-engine exp + fp8 DoubleRow attn@v + Wo folded:

  Host packs per head:  wq' = A5*[Wq; bq]  (A5 = 4/ln2, the e5m2
  Schraudolph constant, folded into q so the DVE exp needs no mult),
  wk' = [Wk; bk],  wv' = [Wv @ Wo_h; bv @ Wo_h | e_ones | 0]  (Wo
  folded into v, so attn@v directly accumulates the per-head output
  numerator and the ones column accumulates the softmax denominator).
  All weights and x^T ship as fp16.

  Per core: qT/kT = fp16 projections (PE, fp32 PSUM, stored fp16).
  v blocks -> fp8e4m3, packed per key-chunk PAIR as [128, 2, 66] for
  DoubleRow matmuls.

  Flash loop over 4 query-quarters x 16 key-chunk pairs (256 keys):
    scores  sp = kT_chunk^T @ qT        [128, 1024] PSUM   (PE, fp16)
    exp     alternates per pair between two engines:
      ACT:  et = e4m3( exp(sp/A5 - 3.6) )          (HW exp table)
      DVE:  et = bitcast_e5m2( int8( max(sp,-B5) + B5 ) )
            -- Schraudolph bit-trick exp: sp is A5*score, so
            t = (score-3.6)*A5 + 60 is the e5m2 bit pattern of
            ~exp(score-3.6); max() clamps the (negligible-mass)
            underflow below score ~ -6.8 to +0.0.
      The -3.6 bias keeps e4m3 in range and cancels in softmax.
    attn@v  2 DoubleRow fp8 matmuls per pair (0.5 cyc/col, K=256):
      bacc[66, 1024] += vab[128,2,66]^T (x) et[128,2,512]
      row 64 of bacc = softmax denominator via the ones column.
  Quarter tail: bacc -> SBUF (split ACT/DVE) -> DMA yt (+ rs row).
  Host: out = sum_h yt_h * (SCALE / rs_h) + bo.

  AV matmuls are emitted one pair late so the in-order PE never waits
  on exp; sp PSUM tiles are freed by exp itself (AV reads SBUF et).
  Engine-balance: ~9/16 pairs on ACT, 7/16 on DVE (plus DVE's copy
  background).  PE ~74us, ACT/DVE ~77us theoretical.

Numerics (numpy-sim of this exact scheme): rel err ~7.7e-3 vs the
2e-2 gate (e5m2 Schraudolph ~5.9e-3 alone; e4m3 exact-exp ~4.8e-3).
"""

import numpy as np

N = 4096
E = 64
H = 8
SCALE = 1.0 / E**0.5
NCORES = 8
W = 1024          # n-quarter width
NQ = N // W       # 4 quarters
NS = W // 512     # 512-wide matmul slices per quarter
NJ = N // 128     # 32 key chunks
NP = NJ // 2      # 16 key-chunk pairs (256 keys each)

A5 = 4.0 / np.log(2.0)          # e5m2 Schraudolph scale (folded into wq)
# exp bias: exp(s+EB); cancels in softmax.  Sized so the largest
# per-head score (9.16) stays under IEEE-e4m3's 240 max: e^(9.16-4.1)
# = 158, with ~1.5x margin for fp16 score error.
EB = -4.1
B5 = 60.0 + EB * A5              # e5m2 exponent-bias term (sp domain)

# per-pair exp engine pattern ('A' = ACT exact exp -> e4m3, 'D' = DVE
# Schraudolph -> e5m2); 17:15 over 32 pairs balances ACT vs DVE+copies
# (timeline-sim sweep: 17:15 beats 16:16 and 18:14)
def _mk_pattern(nA, nD):
    pat, a, d = [], 0, 0
    for _ in range(nA + nD):
        if a * nD <= d * nA and a < nA:
            pat.append('A')
            a += 1
        else:
            pat.append('D')
            d += 1
    return pat


PATTERN = _mk_pattern(17, 15)

_CACHE = {}


def _build_program_v2(reps=1, pattern=None, vab_dt="e4", av_mode="fp8",
                      defer_tail=True, c_r=0.0, exp_split=None):
    """vab_dt: 'e4' (all pairs read e4m3 v), 'e5' (all e5m2), or 'both'
    (A pairs read an e4m3 vab, D pairs an e5m2 vab -- avoids the
    mixed-dtype DoubleRow matmul).  av_mode: 'fp8' (DoubleRow) or
    'bf16' (debug: bf16 et/v, regular matmuls).  exp_split: if set (col
    count), EVERY chunk's exp is split: ACT exact-exps cols [0, split)
    to e5m2 while DVE Schraudolph-exps cols [split, W) -- both engines
    work on every chunk concurrently, the sp tile frees earlier (no PE
    stall on the spool ring), and et is uniformly e5m2."""
    pattern = list(PATTERN if pattern is None else pattern)
    key = ("v2", reps, tuple(pattern), vab_dt, av_mode, defer_tail, c_r,
           exp_split)
    if key in _CACHE:
        return _CACHE[key]

    from contextlib import ExitStack

    import concourse.tile as tile
    from concourse import bacc as bacc_mod, mybir

    f32 = mybir.dt.float32
    f16 = mybir.dt.float16
    f8e4 = mybir.dt.float8e4
    f8e5 = mybir.dt.float8e5
    bf16 = mybir.dt.bfloat16
    i8 = mybir.dt.int8
    i16 = mybir.dt.int16
    Exp = mybir.ActivationFunctionType.Exp
    Max = mybir.AluOpType.max
    Add = mybir.AluOpType.add
    Mult = mybir.AluOpType.mult
    DR = mybir.MatmulPerfMode.DoubleRow

    nc = bacc_mod.Bacc("TRN2", target_bir_lowering=False, debug=False,
                       num_devices=NCORES)

    xt = nc.dram_tensor("xt", [E + 1, N], f16, kind="ExternalInput").ap()
    # packed per-head weights: [wq*A5 | wk | wv_fold + ones col + pad]
    wp = nc.dram_tensor("wp", [E + 1, 3 * E + 2], f16,
                        kind="ExternalInput").ap()
    yt = nc.dram_tensor("yt", [E, N], f32, kind="ExternalOutput").ap()
    rs = nc.dram_tensor("rs", [1, N], f32, kind="ExternalOutput").ap()

    with tile.TileContext(nc) as tc, ExitStack() as ctx:
        rep_loop = (tc.For_i(0, reps, 1) if reps > 1 else None)
        if rep_loop is not None:
            ctx.enter_context(rep_loop)
        const = ctx.enter_context(tc.tile_pool(name="const", bufs=1))
        spool = ctx.enter_context(tc.tile_pool(name="spool", bufs=3,
                                               space="PSUM"))
        bpool = ctx.enter_context(tc.tile_pool(name="bpool", bufs=1,
                                               space="PSUM"))
        epool = ctx.enter_context(tc.tile_pool(name="epool", bufs=4))
        opool = ctx.enter_context(tc.tile_pool(name="opool", bufs=2))

        # warm the ACT exp table before any dependency-carrying work
        scratch = const.tile([1, 1], f32, name="scratch")
        nc.gpsimd.memset(scratch[:], 0.0)
        nc.scalar.activation(scratch[:], scratch[:], Exp)
        # per-partition exp-bias operand for the ACT activations
        ebias = const.tile([128, 1], f32, name="ebias")
        nc.gpsimd.memset(ebias[:], float(EB))

        wp_sb = const.tile([E + 1, 3 * E + 2], f16, name="wp_sb")
        nc.sync.dma_start(wp_sb[:], wp[:])
        wq_sb = wp_sb[:, 0 * E:1 * E]
        wk_sb = wp_sb[:, 1 * E:2 * E]
        wv_sb = wp_sb[:, 2 * E:3 * E + 2]      # (65, 66): ones col + pad
        xt_sb = const.tile([E + 1, N], f16, name="xt_sb")
        # xt chunks on the gpsimd queue so they issue in parallel with
        # the wp DMA on the sync queue
        for c in range(NQ):
            nc.gpsimd.dma_start(xt_sb[:, c * W:(c + 1) * W],
                                xt[:, c * W:(c + 1) * W])

        qt_sb = const.tile([E, N], f16, name="qt_sb")   # A5-scaled q^T
        kt_sb = const.tile([E, N], f16, name="kt_sb")
        # v blocks fp8, pair-major, padded to VBLK=80 bytes per chunk
        # so the DoubleRow ldweights i-stride is 16B-aligned
        # (s3_lw_dual_fp8 ISA restriction); col 64 of each block is the
        # ones column (denominator), cols 66..79 are never read
        VBLK = 80
        vab4 = vab5 = vabb = None
        if av_mode == "bf16":
            vabb = const.tile([128, NP * 2 * VBLK], bf16, name="vabb")
        else:
            if vab_dt in ("e4", "both"):
                vab4 = const.tile([128, NP * 2 * VBLK], f8e4, name="vab4")
            if vab_dt in ("e5", "both"):
                vab5 = const.tile([128, NP * 2 * VBLK], f8e5, name="vab5")

        def vab_for(eng):
            if eng == 'A':
                return vab4 if vab4 is not None else vab5
            return vab5 if vab5 is not None else vab4

        # --- setup helpers (dripped through the first quarters) ---
        def proj_units(c, w_sb, t_sb, nm, use_act_copy=False):
            """3 micro-units: 2 matmuls + 1 PSUM->SBUF fp16 copy."""
            st = {}

            def pp():
                if "pp" not in st:
                    st["pp"] = spool.tile([E, W], f32, tag="s",
                                          name=f"{nm}{c}")
                return st["pp"]

            def mm(s):
                sl = slice(s * 512, (s + 1) * 512)
                xsl = xt_sb[:, c * W + s * 512: c * W + (s + 1) * 512]
                nc.tensor.matmul(pp()[:, sl], w_sb[:], xsl,
                                 start=True, stop=True)

            def cp():
                if use_act_copy:
                    nc.scalar.copy(t_sb[:, c * W:(c + 1) * W], pp()[:])
                else:
                    nc.vector.tensor_copy(t_sb[:, c * W:(c + 1) * W], pp()[:])

            return [lambda: mm(0), lambda: mm(1), cp]

        def v_units(g):
            """2 micro-units covering 4 key-chunks (pairs 2g, 2g+1):
            4 matmuls emitting [v|1] blocks, then 1 copy into vab."""
            st = {}

            def vp():
                if "vp" not in st:
                    st["vp"] = spool.tile([128, 4 * (E + 2)], f32, tag="s",
                                          name=f"vp{g}")
                return st["vp"]

            def mm4():
                for u in range(4):
                    mc = g * 4 + u
                    nc.tensor.matmul(
                        vp()[:, u * (E + 2):(u + 1) * (E + 2)],
                        xt_sb[:, mc * 128:(mc + 1) * 128],
                        wv_sb[:], start=True, stop=True)

            def cp():
                # 4 blocks of 66 strided into the 80-wide padded layout
                src = vp()[:].rearrange("p (b w) -> p b w", w=E + 2)
                for vt in (vab4, vab5, vabb):
                    if vt is None:
                        continue
                    dst = vt[:].rearrange("p (b w) -> p b w", w=VBLK)[
                        :, g * 4:(g + 1) * 4, 0:E + 2]
                    nc.vector.tensor_copy(dst, src)

            return [mm4, cp]

        # chunk 0 of q/k emitted up front at 512 granularity (q copies
        # on ACT, k on DVE), then v groups 0-1 (key chunks 0..7)
        qp0 = spool.tile([E, W], f32, tag="s", name="qp0")
        kp0 = spool.tile([E, W], f32, tag="s", name="kp0")
        for s in range(NS):
            sl = slice(s * 512, (s + 1) * 512)
            xsl = xt_sb[:, s * 512:(s + 1) * 512]
            nc.tensor.matmul(qp0[:, sl], wq_sb[:], xsl, start=True, stop=True)
            nc.tensor.matmul(kp0[:, sl], wk_sb[:], xsl, start=True, stop=True)
            nc.scalar.copy(qt_sb[:, sl], qp0[:, sl])
            nc.vector.tensor_copy(kt_sb[:, sl], kp0[:, sl])
        for u in v_units(0) + v_units(1):
            u()

        # Remaining setup dripped 2 micro-units per pair-slot of quarter
        # 0, with explicit slot alignment so each PSUM staging tile's
        # alloc->copy span stays within the spool ring (<= 2 sp allocs
        # between a pp/vp alloc and its copy).  Deadlines (emission
        # order == Tile dependency order):
        #   kt chunk C needed by scores j=8C, i.e. pair-slot 4C;
        #   v group g (pairs 2g, 2g+1) needed by AV(2g) at slot 2g+1;
        #   qt chunk c needed by quarter c's scores.
        kp1 = proj_units(1, wk_sb, kt_sb, "kp")
        kp2 = proj_units(2, wk_sb, kt_sb, "kp")
        kp3 = proj_units(3, wk_sb, kt_sb, "kp")
        qp1 = proj_units(1, wq_sb, qt_sb, "qp")
        qp2 = proj_units(2, wq_sb, qt_sb, "qp")
        qp3 = proj_units(3, wq_sb, qt_sb, "qp")
        v2, v3, v4, v5 = v_units(2), v_units(3), v_units(4), v_units(5)
        v6, v7 = v_units(6), v_units(7)
        # slot -> units, quarter 0 (slot index = pair t)
        drip0 = {
            1: [kp1[0], kp1[1]],
            2: [kp1[2], v2[0]],
            3: [v2[1], v3[0]],
            4: [v3[1], kp2[0]],
            5: [kp2[1], kp2[2]],
            6: [v4[0], v4[1]],
            7: [v5[0], v5[1]],
            8: [kp3[0], kp3[1]],
            9: [kp3[2], v6[0]],
            10: [v6[1], v7[0]],
            11: [v7[1], qp1[0]],
            12: [qp1[1], qp1[2]],
            13: [qp2[0], qp2[1]],
            14: [qp2[2], qp3[0]],
            15: [qp3[1], qp3[2]],
        }

        # --- main flash-attention loop ---
        hold = {"av": None, "tail": None}
        pair_idx = 0
        for c in range(NQ):
            bst = {}

            def bacc(c=c, bst=bst):
                if "b" not in bst:
                    bst["b"] = bpool.tile([E + 2, W], f32, tag="b",
                                          name=f"b{c}")
                return bst["b"]

            for t in range(NP):
                eng = pattern[pair_idx % len(pattern)]
                pair_idx += 1
                if av_mode == "bf16":
                    et_dt = bf16
                elif exp_split is not None:
                    et_dt = f8e5
                else:
                    et_dt = f8e4
                et = epool.tile([128, 2 * W], et_dt, tag="e",
                                name=f"e{c}_{t}")
                for i in range(2):
                    j = 2 * t + i
                    sp = spool.tile([128, W], f32, tag="s",
                                    name=f"sp{c}_{j}")
                    for s in range(NS):
                        sl = slice(s * 512, (s + 1) * 512)
                        nc.tensor.matmul(
                            sp[:, sl],
                            kt_sb[:, j * 128:(j + 1) * 128],
                            qt_sb[:, c * W + s * 512: c * W + (s + 1) * 512],
                            start=True, stop=True)
                    esl = et[:, i * W:(i + 1) * W]
                    if exp_split is not None and av_mode != "bf16":
                        sa = exp_split
                        nc.scalar.activation(esl[:, 0:sa], sp[:, 0:sa],
                                             Exp, bias=ebias[:],
                                             scale=float(1.0 / A5))
                        nc.vector.tensor_scalar(
                            esl[:, sa:W].bitcast(i8), sp[:, sa:W],
                            float(-B5), float(B5 + c_r), Max, Add)
                    elif eng == 'A':
                        nc.scalar.activation(esl, sp[:], Exp,
                                             bias=ebias[:],
                                             scale=float(1.0 / A5))
                    elif av_mode == "bf16":
                        # bf16 Schraudolph: t = sp*(A7/A5) + B16
                        A7 = 2.0**7 / np.log(2.0)
                        B16 = 16256.0 + EB * A7
                        nc.vector.tensor_scalar(
                            esl.bitcast(i16), sp[:],
                            float(A7 / A5), float(B16), Mult, Add)
                    else:
                        nc.vector.tensor_scalar(
                            esl.bitcast(i8), sp[:],
                            float(-B5), float(B5 + c_r), Max, Add)

                def emit_av(t=t, et=et, eng=eng, bacc=bacc):
                    if av_mode == "bf16":
                        vt_r = vabb[:].rearrange("p (t i m) -> p t i m",
                                                 i=2, m=VBLK)
                        for i in range(2):
                            for h in range(NS):
                                nc.tensor.matmul(
                                    bacc()[:, h * 512:(h + 1) * 512],
                                    vt_r[:, t, i, 0:E + 2],
                                    et[:, i * W + h * 512:
                                       i * W + (h + 1) * 512],
                                    start=(t == 0 and i == 0),
                                    stop=(t == NP - 1 and i == 1))
                        return
                    if exp_split is not None:
                        rhs_t = et[:]          # uniformly e5m2
                    else:
                        rhs_t = et[:] if eng == 'A' else et[:].bitcast(f8e5)
                    rhs_r = rhs_t.rearrange("p (i n) -> p i n", i=2)
                    vt = vab_for(eng)
                    vt_r = vt[:].rearrange("p (t i m) -> p t i m",
                                           i=2, m=VBLK)
                    for h in range(NS):
                        nc.tensor.matmul(
                            bacc()[:, h * 512:(h + 1) * 512],
                            vt_r[:, t, :, 0:E + 2],
                            rhs_r[:, :, h * 512:(h + 1) * 512],
                            start=(t == 0), stop=(t == NP - 1),
                            perf_mode=DR)

                if t == NP - 1:
                    if c < NQ - 1 and not defer_tail:
                        if hold["av"] is not None:
                            hold["av"]()
                            hold["av"] = None
                        emit_av()
                        yo = opool.tile([E + 1, W], f32, tag="y",
                                        name=f"yo{c}")
                        nc.scalar.copy(yo[:, 0:512],
                                       bacc()[0:E + 1, 0:512])
                        nc.vector.tensor_copy(yo[:, 512:1024],
                                              bacc()[0:E + 1, 512:1024])
                        nc.sync.dma_start(yt[:, c * W:(c + 1) * W],
                                          yo[0:E, :])
                        nc.gpsimd.dma_start(rs[0:1, c * W:(c + 1) * W],
                                            yo[E:E + 1, :])
                    elif c < NQ - 1:
                        # flush pair NP-2's deferred AV first
                        if hold["av"] is not None:
                            hold["av"]()
                            hold["av"] = None

                        # defer last AV into the next quarter's pair-0
                        # slot; the bacc->yo copies must be emitted there
                        # too (before pair 1 reallocates the bpool slot),
                        # only the DMAs ride one slot later
                        def make_last(c=c, emit_av=emit_av, bacc=bacc):
                            def last():
                                emit_av()
                                yo = opool.tile([E + 1, W], f32,
                                                tag="y", name=f"yo{c}")
                                nc.scalar.copy(yo[:, 0:512],
                                               bacc()[0:E + 1, 0:512])
                                nc.vector.tensor_copy(
                                    yo[:, 512:1024],
                                    bacc()[0:E + 1, 512:1024])

                                def tail():
                                    nc.sync.dma_start(
                                        yt[:, c * W:(c + 1) * W],
                                        yo[0:E, :])
                                    nc.gpsimd.dma_start(
                                        rs[0:1, c * W:(c + 1) * W],
                                        yo[E:E + 1, :])

                                hold["tail"] = tail
                            return last

                        hold["av"] = make_last()
                    else:
                        # final quarter: emit everything now
                        if hold["av"] is not None:
                            hold["av"]()
                            hold["av"] = None
                        emit_av()
                        yo = opool.tile([E + 1, W], f32, tag="y",
                                        name=f"yo{c}")
                        nc.vector.tensor_copy(yo[:, 0:512],
                                              bacc()[0:E + 1, 0:512])
                        nc.scalar.copy(yo[:, 512:1024],
                                       bacc()[0:E + 1, 512:1024])
                        nc.sync.dma_start(yt[:, c * W:(c + 1) * W],
                                          yo[0:E, :])
                        nc.gpsimd.dma_start(rs[0:1, c * W:(c + 1) * W],
                                            yo[E:E + 1, :])
                else:
                    # AV deferred by one pair so PE never waits on exp
                    if hold["av"] is not None:
                        hold["av"]()
                    hold["av"] = emit_av

                if t == 1 and hold["tail"] is not None:
                    hold["tail"]()
                    hold["tail"] = None
                if c == 0:
                    for u in drip0.get(t, ()):
                        u()

    nc.compile()
    _CACHE[key] = nc
    return nc


def _build_program_v3(reps=1, pattern=None, c_r=0.0, ablate="full",
                      qk_dt="f16"):
    """W=512 restructure: 8 query-groups of 512, 1-bank PSUM tiles.

    PSUM: 6 x sp[128,512] (deep score ring, decouples the two exp
    engines) + 2 x bacc[66,512] (double-buffered -- no quarter-boundary
    deferral).  Exp: one instruction per key-chunk per group, engine by
    PATTERN at pair granularity (A-pairs e4m3 exact exp, D-pairs e5m2
    Schraudolph).  attn@v: one DoubleRow matmul per pair per group."""
    pattern = list(PATTERN if pattern is None else pattern)
    key = ("v3", reps, tuple(pattern), c_r, ablate, qk_dt)
    if key in _CACHE:
        return _CACHE[key]

    from contextlib import ExitStack

    import concourse.tile as tile
    from concourse import bacc as bacc_mod, mybir

    f32 = mybir.dt.float32
    f16 = mybir.dt.float16
    f8e4 = mybir.dt.float8e4
    f8e5 = mybir.dt.float8e5
    i8 = mybir.dt.int8
    Exp = mybir.ActivationFunctionType.Exp
    Max = mybir.AluOpType.max
    Add = mybir.AluOpType.add
    DR = mybir.MatmulPerfMode.DoubleRow

    G = 512                # group width
    NG = N // G            # 8 groups

    nc = bacc_mod.Bacc("TRN2", target_bir_lowering=False, debug=False,
                       num_devices=NCORES)

    xt = nc.dram_tensor("xt", [E + 1, N], f16, kind="ExternalInput").ap()
    wp = nc.dram_tensor("wp", [E + 1, 3 * E + 2], f16,
                        kind="ExternalInput").ap()
    yt = nc.dram_tensor("yt", [E, N], f32, kind="ExternalOutput").ap()
    rs = nc.dram_tensor("rs", [1, N], f32, kind="ExternalOutput").ap()

    with tile.TileContext(nc) as tc, ExitStack() as ctx:
        rep_loop = (tc.For_i(0, reps, 1) if reps > 1 else None)
        if rep_loop is not None:
            ctx.enter_context(rep_loop)
        const = ctx.enter_context(tc.tile_pool(name="const", bufs=1))
        spool = ctx.enter_context(tc.tile_pool(name="spool", bufs=6,
                                               space="PSUM"))
        bpool = ctx.enter_context(tc.tile_pool(name="bpool", bufs=2,
                                               space="PSUM"))
        epool = ctx.enter_context(tc.tile_pool(name="epool", bufs=6))
        opool = ctx.enter_context(tc.tile_pool(name="opool", bufs=3))

        scratch = const.tile([1, 1], f32, name="scratch")
        nc.gpsimd.memset(scratch[:], 0.0)
        nc.scalar.activation(scratch[:], scratch[:], Exp)
        ebias = const.tile([128, 1], f32, name="ebias")
        nc.gpsimd.memset(ebias[:], float(EB))

        wp_sb = const.tile([E + 1, 3 * E + 2], f16, name="wp_sb")
        nc.sync.dma_start(wp_sb[:], wp[:])
        wq_sb = wp_sb[:, 0 * E:1 * E]
        wk_sb = wp_sb[:, 1 * E:2 * E]
        wv_sb = wp_sb[:, 2 * E:3 * E + 2]
        xt_sb = const.tile([E + 1, N], f16, name="xt_sb")
        for c in range(4):
            nc.gpsimd.dma_start(xt_sb[:, c * 1024:(c + 1) * 1024],
                                xt[:, c * 1024:(c + 1) * 1024])

        bf16 = mybir.dt.bfloat16
        qkt = f16 if qk_dt == "f16" else bf16
        qt_sb = const.tile([E, N], qkt, name="qt_sb")   # A5-scaled q^T
        kt_sb = const.tile([E, N], qkt, name="kt_sb")
        VBLK = 80
        vab = const.tile([128, NP * 2 * VBLK], f8e4, name="vab")
        vab_r = vab[:].rearrange("p (t i m) -> p t i m", i=2, m=VBLK)

        # --- setup helpers: 512-grain units (each mm+cp adjacent) ---
        def proj_units(c, w_sb, t_sb, nm, on_act):
            """4 micro-units: (mm, cp) x 2 halves of a 1024-chunk."""
            units = []
            for hf in range(2):
                st = {}

                def mk(hf=hf, st=st):
                    def mm():
                        st["pp"] = spool.tile([E, G], f32, tag="s",
                                              name=f"{nm}{c}_{hf}")
                        sl = slice(c * 1024 + hf * G, c * 1024 + (hf + 1) * G)
                        nc.tensor.matmul(st["pp"][:], w_sb[:], xt_sb[:, sl],
                                         start=True, stop=True)

                    def cp():
                        sl = slice(c * 1024 + hf * G, c * 1024 + (hf + 1) * G)
                        if on_act:
                            nc.scalar.copy(t_sb[:, sl], st["pp"][:])
                        else:
                            nc.vector.tensor_copy(t_sb[:, sl], st["pp"][:])
                    return [mm, cp]

                units += mk()
            return units

        def v_units(g):
            st = {}

            def mm4():
                st["vp"] = spool.tile([128, 4 * (E + 2)], f32, tag="s",
                                      name=f"vp{g}")
                for u in range(4):
                    mc = g * 4 + u
                    nc.tensor.matmul(
                        st["vp"][:, u * (E + 2):(u + 1) * (E + 2)],
                        xt_sb[:, mc * 128:(mc + 1) * 128],
                        wv_sb[:], start=True, stop=True)

            def cp():
                src = st["vp"][:].rearrange("p (b w) -> p b w", w=E + 2)
                dst = vab[:].rearrange("p (b w) -> p b w", w=VBLK)[
                    :, g * 4:(g + 1) * 4, 0:E + 2]
                nc.vector.tensor_copy(dst, src)

            return [mm4, cp]

        # chunk 0 of q/k up front at 512 grain + v groups 0-1
        for hf in range(2):
            sl = slice(hf * G, (hf + 1) * G)
            qp0 = spool.tile([E, G], f32, tag="s", name=f"qp0_{hf}")
            nc.tensor.matmul(qp0[:], wq_sb[:], xt_sb[:, sl],
                             start=True, stop=True)
            nc.scalar.copy(qt_sb[:, sl], qp0[:])
            kp0 = spool.tile([E, G], f32, tag="s", name=f"kp0_{hf}")
            nc.tensor.matmul(kp0[:], wk_sb[:], xt_sb[:, sl],
                             start=True, stop=True)
            nc.vector.tensor_copy(kt_sb[:, sl], kp0[:])
        for u in v_units(0) + v_units(1):
            u()

        kp1 = proj_units(1, wk_sb, kt_sb, "kp", True)
        kp2 = proj_units(2, wk_sb, kt_sb, "kp", False)
        kp3 = proj_units(3, wk_sb, kt_sb, "kp", True)
        qp1 = proj_units(1, wq_sb, qt_sb, "qp", False)
        qp2 = proj_units(2, wq_sb, qt_sb, "qp", True)
        qp3 = proj_units(3, wq_sb, qt_sb, "qp", False)
        v2, v3, v4, v5 = v_units(2), v_units(3), v_units(4), v_units(5)
        v6, v7 = v_units(6), v_units(7)
        # group-0/1 slot -> units; deadlines: kt chunk C by slot 4C-1,
        # v group g by slot 2g, qt chunk c by group 2c
        drip = {
            (0, 1): kp1[0:2], (0, 2): kp1[2:4],
            (0, 3): v2,
            (0, 4): kp2[0:2], (0, 5): kp2[2:4],
            (0, 6): v3,
            (0, 7): v4,
            (0, 8): kp3[0:2], (0, 9): kp3[2:4],
            (0, 10): v5,
            (0, 11): v6,
            (0, 12): v7,
            (0, 13): qp1[0:2], (0, 14): qp1[2:4],
            (0, 15): qp2[0:2],
            (1, 1): qp2[2:4],
            (1, 3): qp3[0:2],
            (1, 5): qp3[2:4],
        }

        # --- main loop: 8 groups x 16 pairs ---
        hold = {"av": None, "tail": None}
        pair_idx = 0
        for g in range(NG):
            bacc_t = bpool.tile([E + 2, G], f32, tag="b", name=f"b{g}")
            for t in range(NP):
                eng = pattern[pair_idx % len(pattern)]
                pair_idx += 1
                et = epool.tile([128, 2 * G], f8e4, tag="e",
                                name=f"e{g}_{t}")
                for i in range(2):
                    j = 2 * t + i
                    if ablate == "empty":
                        continue
                    sp = spool.tile([128, G], f32, tag="s",
                                    name=f"sp{g}_{j}")
                    jj = 0 if ablate == "scores1w" else j
                    nc.tensor.matmul(
                        sp[:],
                        kt_sb[:, jj * 128:(jj + 1) * 128],
                        qt_sb[:, g * G:(g + 1) * G],
                        start=True, stop=True)
                    if ablate in ("scores", "scores1w"):
                        continue
                    esl = et[:, i * G:(i + 1) * G]
                    if eng == 'A':
                        nc.scalar.activation(esl, sp[:], Exp,
                                             bias=ebias[:],
                                             scale=float(1.0 / A5))
                    else:
                        nc.vector.tensor_scalar(
                            esl.bitcast(i8), sp[:],
                            float(-B5), float(B5 + c_r), Max, Add)

                def emit_av(t=t, et=et, eng=eng, bacc_t=bacc_t):
                    if ablate in ("empty", "scores", "scores1w", "noav"):
                        return
                    rhs_t = et[:] if eng == 'A' else et[:].bitcast(f8e5)
                    rhs_r = rhs_t.rearrange("p (i n) -> p i n", i=2)
                    nc.tensor.matmul(
                        bacc_t[:], vab_r[:, t, :, 0:E + 2], rhs_r[:],
                        start=(t == 0), stop=(t == NP - 1),
                        perf_mode=DR)

                def make_tail(g=g, bacc_t=bacc_t):
                    if ablate != "full":
                        return lambda: None

                    def tail():
                        yo = opool.tile([E + 1, G], f32, tag="y",
                                        name=f"yo{g}")
                        if g % 2 == 0:
                            nc.scalar.copy(yo[:], bacc_t[0:E + 1, :])
                        else:
                            nc.vector.tensor_copy(yo[:], bacc_t[0:E + 1, :])
                        nc.sync.dma_start(yt[:, g * G:(g + 1) * G],
                                          yo[0:E, :])
                        nc.gpsimd.dma_start(rs[0:1, g * G:(g + 1) * G],
                                            yo[E:E + 1, :])
                    return tail

                # flush the deferred AV (pair t-1, or the previous
                # group's pair 15 at t==0), then its tail if any; bacc
                # double-buffering lets the new group's AVs overlap
                if hold["av"] is not None:
                    hold["av"]()
                    hold["av"] = None
                if hold["tail"] is not None:
                    hold["tail"]()
                    hold["tail"] = None
                if t < NP - 1:
                    hold["av"] = emit_av
                elif g < NG - 1:
                    hold["av"] = emit_av
                    hold["tail"] = make_tail()
                else:
                    emit_av()
                    make_tail()()

                for u in drip.get((g, t), ()):
                    u()

    nc.compile()
    _CACHE[key] = nc
    return nc


def _build_program_v4(reps=1, pattern=None, c_r=0.0):
    """v4: K-padded scores + 1024-grain exps + defer-2 AVs.

    HW probe findings this encodes: an fp16 matmul with a 64-row
    stationary runs at 2 cycles/col, but 128 rows runs at 1 cycle/col
    -- so qt/kt live on all 128 partitions with rows 64..127 zeroed
    (host-padded xt/wp; gpsimd memsets for qt/kt) and every score
    matmul contracts over K=128.  DR fp8 attn@v measures at its full
    0.5 cyc/col (106.7ns per [66,512]); its apparent cost in earlier
    builds was dependency stalls, so AVs are deferred TWO pairs (~2.1us
    of exp slack).  Exps stay at [128,1024] grain (128 instructions:
    lowest ACT/DVE busy, ~78us balanced) -- the PE throttling on the
    3-deep sp ring is harmless since there is no p-state penalty."""
    pattern = list(PATTERN if pattern is None else pattern)
    key = ("v4", reps, tuple(pattern), c_r)
    if key in _CACHE:
        return _CACHE[key]

    from contextlib import ExitStack

    import concourse.tile as tile
    from concourse import bacc as bacc_mod, mybir

    f32 = mybir.dt.float32
    f16 = mybir.dt.float16
    f8e4 = mybir.dt.float8e4
    f8e5 = mybir.dt.float8e5
    i8 = mybir.dt.int8
    Exp = mybir.ActivationFunctionType.Exp
    Max = mybir.AluOpType.max
    Add = mybir.AluOpType.add
    DR = mybir.MatmulPerfMode.DoubleRow

    nc = bacc_mod.Bacc("TRN2", target_bir_lowering=False, debug=False,
                       num_devices=NCORES)

    # xt rows: 0-63 x^T, 64 ones, 65-127 zeros (K=128 padding)
    xt = nc.dram_tensor("xt", [128, N], f16, kind="ExternalInput").ap()
    wp = nc.dram_tensor("wp", [128, 3 * E + 2], f16,
                        kind="ExternalInput").ap()
    yt = nc.dram_tensor("yt", [E, N], f32, kind="ExternalOutput").ap()
    rs = nc.dram_tensor("rs", [1, N], f32, kind="ExternalOutput").ap()

    with tile.TileContext(nc) as tc, ExitStack() as ctx:
        rep_loop = (tc.For_i(0, reps, 1) if reps > 1 else None)
        if rep_loop is not None:
            ctx.enter_context(rep_loop)
        const = ctx.enter_context(tc.tile_pool(name="const", bufs=1))
        spool = ctx.enter_context(tc.tile_pool(name="spool", bufs=3,
                                               space="PSUM"))
        bpool = ctx.enter_context(tc.tile_pool(name="bpool", bufs=1,
                                               space="PSUM"))
        epool = ctx.enter_context(tc.tile_pool(name="epool", bufs=6))
        opool = ctx.enter_context(tc.tile_pool(name="opool", bufs=2))

        scratch = const.tile([1, 1], f32, name="scratch")
        nc.gpsimd.memset(scratch[:], 0.0)
        nc.scalar.activation(scratch[:], scratch[:], Exp)
        ebias = const.tile([128, 1], f32, name="ebias")
        nc.gpsimd.memset(ebias[:], float(EB))

        wp_sb = const.tile([128, 3 * E + 2], f16, name="wp_sb")
        nc.sync.dma_start(wp_sb[:], wp[:])
        wq_sb = wp_sb[:, 0 * E:1 * E]
        wk_sb = wp_sb[:, 1 * E:2 * E]
        wv_sb = wp_sb[:, 2 * E:3 * E + 2]
        xt_sb = const.tile([128, N], f16, name="xt_sb")
        qt_sb = const.tile([128, N], f16, name="qt_sb")   # A5-scaled q^T
        kt_sb = const.tile([128, N], f16, name="kt_sb")
        for c in range(NQ):
            sl = slice(c * W, (c + 1) * W)
            nc.gpsimd.dma_start(xt_sb[:, sl], xt[:, sl])
            # zero the contraction-padding rows (0 x NaN-garbage = NaN,
            # so BOTH operands' pad rows must be real zeros)
            nc.gpsimd.memset(qt_sb[64:128, sl], 0.0)
            nc.gpsimd.memset(kt_sb[64:128, sl], 0.0)

        VBLK = 80
        vab = const.tile([128, NP * 2 * VBLK], f8e4, name="vab")
        vab_r = vab[:].rearrange("p (t i m) -> p t i m", i=2, m=VBLK)

        def proj_units(c, w_sb, t_sb, nm, use_act_copy=False):
            st = {}

            def pp():
                if "pp" not in st:
                    st["pp"] = spool.tile([E, W], f32, tag="s",
                                          name=f"{nm}{c}")
                return st["pp"]

            def mm(s):
                sl = slice(s * 512, (s + 1) * 512)
                xsl = xt_sb[:, c * W + s * 512: c * W + (s + 1) * 512]
                nc.tensor.matmul(pp()[:, sl], w_sb[:], xsl,
                                 start=True, stop=True)

            def cp():
                if use_act_copy:
                    nc.scalar.copy(t_sb[0:E, c * W:(c + 1) * W], pp()[:])
                else:
                    nc.vector.tensor_copy(t_sb[0:E, c * W:(c + 1) * W],
                                          pp()[:])

            return [lambda: mm(0), lambda: mm(1), cp]

        def v_units(g):
            st = {}

            def vp():
                if "vp" not in st:
                    st["vp"] = spool.tile([128, 4 * (E + 2)], f32, tag="s",
                                          name=f"vp{g}")
                return st["vp"]

            def mm4():
                for u in range(4):
                    mc = g * 4 + u
                    nc.tensor.matmul(
                        vp()[:, u * (E + 2):(u + 1) * (E + 2)],
                        xt_sb[:, mc * 128:(mc + 1) * 128],
                        wv_sb[:], start=True, stop=True)

            def cp():
                src = vp()[:].rearrange("p (b w) -> p b w", w=E + 2)
                dst = vab[:].rearrange("p (b w) -> p b w", w=VBLK)[
                    :, g * 4:(g + 1) * 4, 0:E + 2]
                nc.vector.tensor_copy(dst, src)

            return [mm4, cp]

        # chunk 0 of q/k up front + v groups 0-1
        qp0 = spool.tile([E, W], f32, tag="s", name="qp0")
        kp0 = spool.tile([E, W], f32, tag="s", name="kp0")
        for s in range(NS):
            sl = slice(s * 512, (s + 1) * 512)
            xsl = xt_sb[:, s * 512:(s + 1) * 512]
            nc.tensor.matmul(qp0[:, sl], wq_sb[:], xsl, start=True, stop=True)
            nc.tensor.matmul(kp0[:, sl], wk_sb[:], xsl, start=True, stop=True)
            nc.scalar.copy(qt_sb[0:E, sl], qp0[:, sl])
            nc.vector.tensor_copy(kt_sb[0:E, sl], kp0[:, sl])
        for u in v_units(0) + v_units(1):
            u()

        kp1 = proj_units(1, wk_sb, kt_sb, "kp")
        kp2 = proj_units(2, wk_sb, kt_sb, "kp")
        kp3 = proj_units(3, wk_sb, kt_sb, "kp")
        qp1 = proj_units(1, wq_sb, qt_sb, "qp", True)
        qp2 = proj_units(2, wq_sb, qt_sb, "qp")
        qp3 = proj_units(3, wq_sb, qt_sb, "qp", True)
        v2u, v3u = v_units(2), v_units(3)
        v4u, v5u = v_units(4), v_units(5)
        v6u, v7u = v_units(6), v_units(7)
        # quarter-0 drip (slot = pair t); deadlines: kt chunk C by pair
        # 4C, v group g by AV(2g) at slot 2g+2, qt chunk c by quarter c;
        # proj staging tiles must free within ~2 slots (3-deep spool)
        drip0 = {
            1: [kp1[0], kp1[1]],
            2: [kp1[2], v2u[0]],
            3: [v2u[1], v3u[0]],
            4: [v3u[1], kp2[0]],
            5: [kp2[1], kp2[2]],
            6: [v4u[0], v4u[1]],
            7: [v5u[0], v5u[1]],
            8: [kp3[0], kp3[1]],
            9: [kp3[2], v6u[0]],
            10: [v6u[1], v7u[0]],
            11: [v7u[1], qp1[0]],
            12: [qp1[1], qp1[2]],
            13: [qp2[0], qp2[1]],
            14: [qp2[2], qp3[0]],
            15: [qp3[1], qp3[2]],
        }

        # --- main loop: 4 quarters x 16 pairs, AVs deferred 2 pairs ---
        avq = []   # deque of (emit_fn, tail_fn|None)
        pair_idx = 0
        for c in range(NQ):
            bst = {}

            def bacc(c=c, bst=bst):
                if "b" not in bst:
                    bst["b"] = bpool.tile([E + 2, W], f32, tag="b",
                                          name=f"b{c}")
                return bst["b"]

            for t in range(NP):
                eng = pattern[pair_idx % len(pattern)]
                pair_idx += 1
                et = epool.tile([128, 2 * W], f8e4, tag="e",
                                name=f"e{c}_{t}")
                for i in range(2):
                    j = 2 * t + i
                    sp = spool.tile([128, W], f32, tag="s",
                                    name=f"sp{c}_{j}")
                    for s in range(NS):
                        sl = slice(s * 512, (s + 1) * 512)
                        nc.tensor.matmul(
                            sp[:, sl],
                            kt_sb[:, j * 128:(j + 1) * 128],
                            qt_sb[:, c * W + s * 512: c * W + (s + 1) * 512],
                            start=True, stop=True)
                    esl = et[:, i * W:(i + 1) * W]
                    if eng == 'A':
                        nc.scalar.activation(esl, sp[:], Exp,
                                             bias=ebias[:],
                                             scale=float(1.0 / A5))
                    else:
                        nc.vector.tensor_scalar(
                            esl.bitcast(i8), sp[:],
                            float(-B5), float(B5 + c_r), Max, Add)

                def emit_av(t=t, et=et, eng=eng, bacc=bacc):
                    rhs_t = et[:] if eng == 'A' else et[:].bitcast(f8e5)
                    rhs_r = rhs_t.rearrange("p (i n) -> p i n", i=2)
                    for h in range(NS):
                        nc.tensor.matmul(
                            bacc()[:, h * 512:(h + 1) * 512],
                            vab_r[:, t, :, 0:E + 2],
                            rhs_r[:, :, h * 512:(h + 1) * 512],
                            start=(t == 0), stop=(t == NP - 1),
                            perf_mode=DR)

                def make_tail(c=c, bacc=bacc):
                    def tail():
                        yo = opool.tile([E + 1, W], f32, tag="y",
                                        name=f"yo{c}")
                        nc.scalar.copy(yo[:, 0:512], bacc()[0:E + 1, 0:512])
                        nc.vector.tensor_copy(yo[:, 512:1024],
                                              bacc()[0:E + 1, 512:1024])
                        nc.sync.dma_start(yt[:, c * W:(c + 1) * W],
                                          yo[0:E, :])
                        nc.gpsimd.dma_start(rs[0:1, c * W:(c + 1) * W],
                                            yo[E:E + 1, :])
                    return tail

                avq.append((emit_av, make_tail() if t == NP - 1 else None))
                while len(avq) > 2:
                    fn, tail = avq.pop(0)
                    fn()
                    if tail is not None:
                        tail()

                for u in (drip0.get(t, ()) if c == 0 else ()):
                    u()

        while avq:
            fn, tail = avq.pop(0)
            fn()
            if tail is not None:
                tail()

    nc.compile()
    _CACHE[key] = nc
    return nc


def _build_program(reps=1, **kw):
    return _build_program_v4(reps=reps, **kw)


def _run(in_maps, trace=False, trace_cores=None):
    from concourse.bass_utils import run_bass_kernel_spmd

    nc = _build_program()
    return run_bass_kernel_spmd(nc, in_maps, list(range(NCORES)),
                                trace=trace, trace_cores=trace_cores)


def make_in_maps(x, Wq, bq, Wk, bk, Wv, bv, Wo, bo):
    x = np.asarray(x, np.float32)
    Wq, bq = np.asarray(Wq, np.float64), np.asarray(bq, np.float64)
    Wk, bk = np.asarray(Wk, np.float64), np.asarray(bk, np.float64)
    Wv, bv = np.asarray(Wv, np.float64), np.asarray(bv, np.float64)
    Wo = np.asarray(Wo, np.float64)

    # rows 0-63: x^T; row 64: ones (bias selector); rows 65-127: zeros
    # (K=128 contraction padding -- see _build_program_v4)
    xt_aug = np.zeros((128, N), np.float16)
    xt_aug[:E] = x.T.astype(np.float16)
    xt_aug[E] = 1.0

    in_maps = []
    for h in range(H):
        Wo_h = Wo[h * E:(h + 1) * E]
        wpack = np.zeros((128, 3 * E + 2), np.float64)
        wpack[:E, 0 * E:1 * E] = Wq[h] * A5
        wpack[E, 0 * E:1 * E] = bq[h] * A5
        wpack[:E, 1 * E:2 * E] = Wk[h]
        wpack[E, 1 * E:2 * E] = bk[h]
        wpack[:E, 2 * E:3 * E] = Wv[h] @ Wo_h
        wpack[E, 2 * E:3 * E] = bv[h] @ Wo_h
        wpack[E, 3 * E] = 1.0            # ones column selector
        in_maps.append({"xt": xt_aug, "wp": wpack.astype(np.float16)})
    return in_maps


def combine_results(results, bo):
    bo = np.asarray(bo, np.float64)
    out = np.zeros((N, E), np.float64)
    for h in range(H):
        yth = results[h]["yt"].astype(np.float64)      # (64, 4096)
        rsh = results[h]["rs"].astype(np.float64)      # (1, 4096)
        out += (yth * (SCALE / rsh)).T
    out += bo
    return out.astype(np.float32)


def kernel(x, Wq, bq, Wk, bk, Wv, bv, Wo, bo):
    in_maps = make_in_maps(x, Wq, bq, Wk, bk, Wv, bv, Wo, bo)
    res = _run(in_maps)
    return combine_results(res.results, bo)
